# revision 18
# baseline (speedup 1.0000x reference)
"""DCNN Trainium kernel v2: grouped conv1 + kmax131 + grouped conv2 + top4 + FC.

All-bf16 value pipeline after conv1 (f32 matmuls): per-row k-max threshold via
bf16 bisection (4x-mode tensor_scalar counts) + exact max8 closer in the bf16
domain; compaction via single-index bf16 local_scatter with idx = cs*msk - 1;
conv bias folded into Act-engine PSUM->SBUF activations; conv2/fc in bf16/f16;
top-4 selection in tanh domain.  Copies/memsets offloaded to Pool engine.
"""
import numpy as np
import concourse.bass as bass
import concourse.tile as tile
from concourse import bacc, mybir

F32 = mybir.dt.float32
F16 = mybir.dt.float16
BF16 = mybir.dt.bfloat16
I16 = mybir.dt.int16
I32 = mybir.dt.int32
U16 = mybir.dt.uint16
OP = mybir.AluOpType
AFT = mybir.ActivationFunctionType
AXL = mybir.AxisListType

S = 256
S1 = 262          # conv1 out length
K1 = 131          # layer-1 keep count
S1P = 264         # padded Vb row stride
H1W = 140         # compact row stride: 4 pad + 132 + 4 pad
S2 = 135          # conv2 out length
S2P = 136
ROUNDS = 8
BIG = 1024.0


def host_prep(x_core, emb, w1, b1, w2, b2, fc_w, SUBB, NSUB):
    B_C = SUBB * NSUB
    assert x_core.shape == (B_C, S)
    toks = np.ascontiguousarray(x_core.reshape(-1).astype(np.int32))
    ncol_s = SUBB * S // 128
    xi = toks.reshape(NSUB, ncol_s, 128).transpose(2, 0, 1).reshape(128, NSUB * ncol_s)

    w1 = np.asarray(w1, np.float32)   # (512, 1, 7)
    b1 = np.asarray(b1, np.float32)
    w2 = np.asarray(w2, np.float32)   # (512, 8, 5)
    b2 = np.asarray(b2, np.float32)
    fc_w = np.asarray(fc_w, np.float32)

    # conv1 lhsT with fold fused: l1[u, gl*7+k, rl*8+f] = w1[(16u+gl)*8+f, 0, k] if rl==(16(u%2)+gl)//2
    u_, gl_, k_, rl_, f_ = np.ix_(np.arange(4), np.arange(16), np.arange(7), np.arange(16), np.arange(8))
    val = w1[(16 * u_ + gl_) * 8 + f_, 0, k_]                       # (4,16,7,16,8)
    sel = (rl_ == (16 * (u_ % 2) + gl_) // 2)
    l1 = (val * sel).astype(np.float32).reshape(4, 112, 128)

    r_ = np.arange(32)
    f8 = np.arange(8)
    bf1 = (b1[(2 * r_[:, None]) * 8 + f8[None, :]] + b1[(2 * r_[:, None] + 1) * 8 + f8[None, :]]).reshape(256)
    bf1_t = np.ascontiguousarray(bf1.reshape(2, 128).T)             # [128, 2]

    q_, k5_, rrl_, ci_, r2l_, f2_ = np.ix_(np.arange(2), np.arange(5), np.arange(16), np.arange(8), np.arange(8), np.arange(16))
    val2 = w2[(16 * q_ + rrl_) * 16 + f2_, ci_, k5_]                # (2,5,16,8,8,16)
    sel2 = (r2l_ == rrl_ // 2)
    l2 = (val2 * sel2).astype(np.float32).reshape(10, 128, 128)
    r2_ = np.arange(16)
    f16_ = np.arange(16)
    bf2 = (b2[(2 * r2_[:, None]) * 16 + f16_[None, :]] + b2[(2 * r2_[:, None] + 1) * 16 + f16_[None, :]]).reshape(256)
    bf2_t = np.ascontiguousarray(bf2.reshape(2, 128).T)             # [128, 2]

    cl_ = np.arange(128)
    fcw = np.stack([fc_w[:, (128 * q + cl_) * 4 + j].T for q in range(2) for j in range(4)], 0)

    s1sq = (w1[:, 0, :] ** 2).sum(1)                                 # (512,)
    var = 0.01 * (s1sq[(2 * r_[:, None]) * 8 + f8[None, :]] + s1sq[(2 * r_[:, None] + 1) * 8 + f8[None, :]])
    sig = np.sqrt(var).astype(np.float32).reshape(2, 128)            # [m, rl*8+f]
    # sg[p, h*16+b] = 0.35 * sig[h, p]
    sg = np.repeat(0.35 * sig.T[:, :, None], SUBB, axis=2).reshape(128, 2 * SUBB)

    def as_bf16(a):
        # numpy has no bf16; emulate via f32 -> truncate-round mantissa
        x = np.asarray(a, np.float32).copy()
        u = x.view(np.uint32)
        u += 0x7FFF + ((u >> 16) & 1)
        return (u & 0xFFFF0000).view(np.float32).astype(np.float32)

    l2_flat = np.ascontiguousarray(l2.transpose(1, 0, 2).reshape(128, 1280))
    fcw_flat = np.ascontiguousarray(fcw.transpose(1, 0, 2).reshape(128, 48))

    return dict(
        x_idx=np.ascontiguousarray(xi),
        emb=np.ascontiguousarray(np.asarray(emb, np.float32)),
        l1=np.ascontiguousarray(l1.transpose(1, 0, 2).reshape(112, 512)),
        bf1=bf1_t,
        l2=l2_flat,
        bf2=bf2_t,
        fcw=fcw_flat,
        sg=np.ascontiguousarray(sg.astype(np.float32)),
        iota8=np.tile(np.arange(8, dtype=np.float32), (128, 1)),
        ident=np.eye(128, dtype=np.float32),
    )


def _win(ap, K, T):
    """[P, W] AP -> [P, K, T] overlapping window view (free steps 1, 1)."""
    l = [list(p) for p in ap.ap]
    assert len(l) == 2 and l[1][0] == 1
    return bass.AP(ap.tensor, ap.offset, [l[0], [1, K], [1, T]])


def build_nc(SUBB, NSUB):
    B_C = SUBB * NSUB
    ncol_s = SUBB * S // 128
    nc = bacc.Bacc("TRN2", target_bir_lowering=False, debug=False)
    D = {}
    D['x_idx'] = nc.dram_tensor("x_idx", [128, NSUB * ncol_s], I32, kind="ExternalInput")
    D['emb'] = nc.dram_tensor("emb", [100000, 64], F32, kind="ExternalInput")
    D['l1'] = nc.dram_tensor("l1", [112, 512], F32, kind="ExternalInput")
    D['bf1'] = nc.dram_tensor("bf1", [128, 2], F32, kind="ExternalInput")
    D['l2'] = nc.dram_tensor("l2", [128, 1280], F32, kind="ExternalInput")
    D['bf2'] = nc.dram_tensor("bf2", [128, 2], F32, kind="ExternalInput")
    D['fcw'] = nc.dram_tensor("fcw", [128, 48], F32, kind="ExternalInput")
    D['sg'] = nc.dram_tensor("sg", [128, 2 * SUBB], F32, kind="ExternalInput")
    D['iota8'] = nc.dram_tensor("iota8", [128, 8], F32, kind="ExternalInput")
    D['ident'] = nc.dram_tensor("ident", [128, 128], F32, kind="ExternalInput")
    out = nc.dram_tensor("out", [6, B_C], F32, kind="ExternalOutput")
    dbg_mu = nc.dram_tensor("dbg_mu", [128, 2 * SUBB], F32, kind="ExternalOutput")
    dbg_tb = nc.dram_tensor("dbg_tb", [128, 2 * SUBB], F32, kind="ExternalOutput")
    dbg_cbl = nc.dram_tensor("dbg_cbl", [128, 2 * SUBB], F32, kind="ExternalOutput")
    dbg_vst = nc.dram_tensor("dbg_vst", [128, 2 * SUBB], F32, kind="ExternalOutput")
    dbg_h1c = nc.dram_tensor("dbg_h1c", [128, SUBB * H1W], F32, kind="ExternalOutput")
    dbg_th4 = nc.dram_tensor("dbg_th4", [128, SUBB * 8], F32, kind="ExternalOutput")
    dbg_th4b = nc.dram_tensor("dbg_th4b", [128, SUBB * 8], F32, kind="ExternalOutput")
    NB2 = 2 * SUBB
    with tile.TileContext(nc) as tc:
        from contextlib import ExitStack
        es = ExitStack()
        cpool = es.enter_context(tc.tile_pool(name="const", bufs=1))
        xtpool = es.enter_context(tc.tile_pool(name="xt", bufs=1))
        gpool = es.enter_context(tc.tile_pool(name="gather", bufs=1))
        ptr = es.enter_context(tc.tile_pool(name="ptr", bufs=2, space="PSUM"))
        icpool = es.enter_context(tc.tile_pool(name="ic", bufs=2))
        ps1pool = es.enter_context(tc.tile_pool(name="ps1", bufs=2, space="PSUM"))
        vspool = es.enter_context(tc.tile_pool(name="vs", bufs=2))
        stpool = es.enter_context(tc.tile_pool(name="st", bufs=1))
        scrpool = es.enter_context(tc.tile_pool(name="scr", bufs=4))
        mpool = es.enter_context(tc.tile_pool(name="mask", bufs=1))
        h1cpool = es.enter_context(tc.tile_pool(name="h1c", bufs=1))
        ps2pool = es.enter_context(tc.tile_pool(name="ps2", bufs=2, space="PSUM"))
        thpool = es.enter_context(tc.tile_pool(name="th", bufs=1))
        th4pool = es.enter_context(tc.tile_pool(name="th4", bufs=2))
        psfc = es.enter_context(tc.tile_pool(name="psfc", bufs=2, space="PSUM"))
        outpool = es.enter_context(tc.tile_pool(name="outp", bufs=1))
        if True:
            ident = cpool.tile([128, 128], F32)
            nc.gpsimd.dma_start(ident[:], D['ident'][:])
            iota8 = cpool.tile([128, 8], F32)
            nc.gpsimd.dma_start(iota8[:], D['iota8'][:])
            sg = cpool.tile([128, NB2], F32)
            nc.gpsimd.dma_start(sg[:], D['sg'][:])
            bf1 = cpool.tile([128, 2], F32)
            nc.gpsimd.dma_start(bf1[:], D['bf1'][:])
            bf2 = cpool.tile([128, 2], F32)
            nc.gpsimd.dma_start(bf2[:], D['bf2'][:])
            w1t = cpool.tile([112, 512], F32)
            nc.gpsimd.dma_start(w1t[:], D['l1'][:])
            w2t = cpool.tile([128, 1280], F32)
            nc.gpsimd.dma_start(w2t[:], D['l2'][:])
            fcw = cpool.tile([128, 48], F32)
            nc.gpsimd.dma_start(fcw[:], D['fcw'][:])
            xidx = cpool.tile([128, NSUB * ncol_s], I32)
            nc.gpsimd.dma_start(xidx[:], D['x_idx'][:])
            zer262 = cpool.tile([128, S1], F16)
            nc.vector.memset(zer262[:], 0.0)
            out_sb = outpool.tile([6, B_C], F32)
            nc.vector.memset(out_sb[:], 0.0)

            MU = stpool.tile([128, NB2], F32, name="MU")
            TA = stpool.tile([128, NB2], F32, name="TA")
            TB = stpool.tile([128, NB2], F32, name="TB")
            PV = stpool.tile([128, NB2], F32, name="PV")
            CC = stpool.tile([128, NB2], F32, name="CC")
            CBl = stpool.tile([128, NB2], F32, name="CBl")
            JJ = stpool.tile([128, NB2], F32, name="JJ")
            VST = stpool.tile([128, NB2], F32, name="VST")
            MX = stpool.tile([128, NB2 * 8], F32, name="MX")
            MX2 = stpool.tile([128, NB2 * 8], F32, name="MX2")
            for t_ in (MU, TA, TB, PV, CC, CBl, JJ, VST, MX, MX2):
                nc.vector.memset(t_[:], 0.0)

            for s in range(NSUB):
                # ---------------- PHASE A: gather + transpose ----------------
                xt = xtpool.tile([64, SUBB * 268], F32, tag="xt", name="xt")
                xt3 = xt[:].rearrange("p (b t) -> p b t", t=268)
                nc.vector.memset(xt3[:, :, 0:6], 0.0)
                nc.vector.memset(xt3[:, :, 262:268], 0.0)
                for i in range(ncol_s):
                    gt = gpool.tile([128, 64], F32, tag="gt", name="gt", bufs=2)
                    nc.gpsimd.indirect_dma_start(
                        out=gt[:], out_offset=None, in_=D['emb'][:],
                        in_offset=bass.IndirectOffsetOnAxis(
                            ap=xidx[:, s * ncol_s + i: s * ncol_s + i + 1], axis=0))
                    pt = ptr.tile([64, 128], F32, tag="pt", name="pt")
                    nc.tensor.transpose(out=pt[:], in_=gt[:], identity=ident[:])
                    off = (i // 2) * 268 + 6 + (i % 2) * 128
                    nc.scalar.activation(xt[:, off:off + 128], pt[:], AFT.Identity)

                # ---------------- PHASE B: conv1 (+fold +bias via Act) -------
                Vs = [vspool.tile([128, SUBB * S1P], F32, tag=f"Vs{h}", name=f"Vs{h}") for h in range(2)]
                for h in range(2):
                    vs3 = Vs[h][:].rearrange("p (b t) -> p b t", t=S1P)
                    nc.vector.memset(vs3[:, :, S1:S1P], 0.0)

                for b in range(SUBB):
                    ics = []
                    for u in range(4):
                        ic = icpool.tile([112, S1], F32, tag=f"ic{u}", name=f"ic{u}")
                        sap = _win(xt[16 * u:16 * u + 16, b * 268: b * 268 + 268], 7, S1)
                        nc.sync.dma_start(ic[:], sap)
                        ics.append(ic)
                    for m in range(2):
                        col = m * SUBB + b
                        p1 = ps1pool.tile([128, S1], F32, tag="p1", name="p1")
                        nc.tensor.matmul(p1[:], w1t[:, 2 * m * 128:(2 * m + 1) * 128], ics[2 * m][:],
                                         start=True, stop=False)
                        nc.tensor.matmul(p1[:], w1t[:, (2 * m + 1) * 128:(2 * m + 2) * 128], ics[2 * m + 1][:],
                                         start=False, stop=True)
                        nc.scalar.activation(Vs[m][:, b * S1P: b * S1P + S1], p1[:],
                                             AFT.Identity, bias=bf1[:, m:m + 1],
                                             accum_out=MU[:, col:col + 1])

                # ---------------- PHASE C: kmax-131 (both h jointly) ---------
                nc.vector.tensor_scalar(PV[:], MU[:], 1.0 / S1, 0.0, OP.mult, OP.add)
                nc.vector.tensor_tensor(TA[:], PV[:], sg[:], OP.subtract)
                nc.vector.tensor_tensor(TB[:], PV[:], sg[:], OP.add)
                for r in range(ROUNDS):
                    nc.vector.tensor_tensor(PV[:], TA[:], TB[:], OP.add)
                    nc.vector.tensor_scalar(PV[:], PV[:], 0.5, 0.0, OP.mult, OP.add)
                    for h in range(2):
                        for b in range(SUBB):
                            col = h * SUBB + b
                            scr = scrpool.tile([128, S1], F32, tag="scr32", name="scr32")
                            nc.vector.tensor_scalar(
                                scr[:], Vs[h][:, b * S1P: b * S1P + S1],
                                PV[:, col:col + 1], 0.0, OP.is_ge, OP.add,
                                accum_out=CC[:, col:col + 1])
                    cond = scrpool.tile([128, NB2], I32, tag="cond", name="cond")
                    condn = scrpool.tile([128, NB2], I32, tag="condn", name="condn")
                    nc.vector.tensor_scalar(cond[:], CC[:], float(K1) - 0.5, 0.0, OP.is_ge, OP.add)
                    nc.vector.tensor_scalar(condn[:], CC[:], float(K1) - 0.5, 0.0, OP.is_lt, OP.add)
                    nc.vector.copy_predicated(TA[:], cond[:], PV[:])
                    nc.vector.copy_predicated(TB[:], condn[:], PV[:])

                # final mask(<TB) + count + closer + compact + conv2, per h
                for h in range(2):
                    mlt = mpool.tile([128, SUBB * S1P], F16, tag="mlt", name="mlt")
                    m3 = mlt[:].rearrange("p (b t) -> p b t", t=S1P)
                    nc.vector.memset(m3[:, :, S1:S1P], 0.0)
                    for b in range(SUBB):
                        col = h * SUBB + b
                        nc.vector.tensor_scalar(
                            mlt[:, b * S1P: b * S1P + S1], Vs[h][:, b * S1P: b * S1P + S1],
                            TB[:, col:col + 1], 0.0, OP.is_lt, OP.add,
                            accum_out=CBl[:, col:col + 1])
                    c0, c1 = h * SUBB, (h + 1) * SUBB
                    nc.vector.tensor_scalar(JJ[:, c0:c1], CBl[:, c0:c1], 1.0,
                                            -(float(S1 - K1 + 1)), OP.mult, OP.add)
                    # exclude >=TB cells by pushing them to ~-BIG (in-place on Vs)
                    aneg = mpool.tile([128, SUBB * S1P], F16, tag="aneg", name="aneg")
                    nc.vector.tensor_scalar(aneg[:], mlt[:], BIG, -BIG, OP.mult, OP.add)
                    msel = mpool.tile([128, SUBB * S1P], F32, tag="mlt", name="msel")
                    nc.vector.tensor_tensor(msel[:], Vs[h][:], aneg[:], OP.add)
                    for b in range(SUBB):
                        col = h * SUBB + b
                        nc.vector.max(MX[:, col * 8: col * 8 + 8], msel[:, b * S1P: b * S1P + S1])
                    # VST = jj-th largest below TB (one-hot via iota8)
                    oh = scrpool.tile([128, SUBB, 8], F32, tag="oh", name="oh")
                    jj_b = JJ[:, c0:c1].unsqueeze(2).broadcast_to([128, SUBB, 8])
                    io_b = iota8[:].unsqueeze(1).broadcast_to([128, SUBB, 8])
                    nc.vector.tensor_tensor(oh[:], io_b, jj_b, OP.is_equal)
                    mx3 = MX[:, c0 * 8: c1 * 8].rearrange("p (c e) -> p c e", e=8)
                    nc.vector.tensor_tensor(oh[:], oh[:], mx3, OP.mult)
                    nc.vector.tensor_reduce(VST[:, c0:c1], oh[:], AXL.X, OP.add)

                    H1C = h1cpool.tile([128, SUBB * H1W], F32, tag=f"H1C{h}", name=f"H1C{h}")
                    nc.gpsimd.memset(H1C[:], 0.0)
                    msk = mpool.tile([128, SUBB * S1P], F16, tag="msk", name="msk")
                    mk3 = msk[:].rearrange("p (b t) -> p b t", t=S1P)
                    nc.vector.memset(mk3[:, :, S1:S1P], 0.0)
                    for b in range(SUBB):
                        col = h * SUBB + b
                        nc.vector.tensor_scalar(
                            msk[:, b * S1P: b * S1P + S1], Vs[h][:, b * S1P: b * S1P + S1],
                            VST[:, col:col + 1], 0.0, OP.is_ge, OP.add)
                    cs = mpool.tile([128, SUBB * S1P], F16, tag="cs", name="cs")
                    cs3 = cs[:].rearrange("p (b t) -> p b t", t=S1P)
                    nc.vector.memset(cs3[:, :, S1:S1P], 0.0)
                    for b in range(SUBB):
                        nc.vector.tensor_tensor_scan(
                            cs[:, b * S1P: b * S1P + S1], msk[:, b * S1P: b * S1P + S1],
                            zer262[:], 0.0, OP.add, OP.add)
                    nc.vector.tensor_tensor(cs[:], cs[:], msk[:], OP.mult)
                    capc = mpool.tile([128, SUBB * S1P], F16, tag="aneg", name="capc")
                    nc.vector.tensor_scalar(capc[:], cs[:], float(K1) + 0.5, 0.0, OP.is_lt, OP.add)
                    nc.vector.tensor_tensor(cs[:], cs[:], capc[:], OP.mult)
                    idxt = mpool.tile([128, SUBB, 2 * S1P], I16, tag="idxt", name="idxt")
                    cs3b = cs[:].rearrange("p (b t) -> p b t", t=S1P)
                    nc.vector.tensor_scalar(idxt[:, :, 0: 2 * S1: 2], cs3b[:, :, 0:S1], 2.0, -2.0, OP.mult, OP.add)
                    nc.vector.tensor_scalar(idxt[:, :, 1: 2 * S1: 2], cs3b[:, :, 0:S1], 2.0, -1.0, OP.mult, OP.add)
                    h1u = H1C[:].bitcast(U16)
                    vsu = Vs[h][:].bitcast(U16)
                    for b in range(SUBB):
                        dst = h1u[:, 2 * (b * H1W + 4): 2 * (b * H1W + 136)]
                        src_ = vsu[:, 2 * b * S1P: 2 * b * S1P + 2 * S1]
                        nc.gpsimd.local_scatter(
                            dst, src_, idxt[:, b, 0: 2 * S1],
                            channels=128, num_elems=264, num_idxs=2 * S1)
                    if s == 0 and h == 0:
                        dbg_h1f = outpool.tile([128, SUBB * H1W], F32, name="dbg_h1f")
                        nc.vector.tensor_copy(dbg_h1f[:], H1C[:])
                        nc.sync.dma_start(dbg_h1c[:], dbg_h1f[:])
                    nc.scalar.activation(H1C[:], H1C[:], AFT.Tanh)

                    # ------------- PHASE D: conv2 + top4 + fc (q == h) -------
                    q = h
                    th = thpool.tile([128, SUBB * S2P], F32, tag="th", name="th")
                    th3 = th[:].rearrange("p (b t) -> p b t", t=S2P)
                    nc.vector.memset(th3[:, :, S2:S2P], 0.0)
                    m2 = mpool.tile([128, SUBB * S2P], F16, tag="msk", name="m2")
                    m23 = m2[:].rearrange("p (b t) -> p b t", t=S2P)
                    nc.vector.memset(m23[:, :, S2:S2P], 0.0)
                    for b in range(SUBB):
                        col = q * SUBB + b
                        p2 = ps2pool.tile([128, S2], F32, tag="p2", name="p2")
                        for k in range(5):
                            nc.tensor.matmul(p2[:], w2t[:, (q * 5 + k) * 128:(q * 5 + k + 1) * 128],
                                             H1C[:, b * H1W + k: b * H1W + k + S2],
                                             start=(k == 0), stop=(k == 4))
                        nc.scalar.activation(th[:, b * S2P: b * S2P + S2], p2[:],
                                             AFT.Tanh, bias=bf2[:, q:q + 1])
                        # top-4 selection on exact f32 psum (monotone under bias+tanh)
                        nc.vector.max(MX2[:, col * 8: col * 8 + 8], p2[:])
                        nc.vector.tensor_scalar(
                            m2[:, b * S2P: b * S2P + S2], p2[:],
                            MX2[:, col * 8 + 3: col * 8 + 4], 0.0, OP.is_ge, OP.add)
                    c2 = mpool.tile([128, SUBB * S2P], F16, tag="cs", name="c2")
                    c23 = c2[:].rearrange("p (b t) -> p b t", t=S2P)
                    nc.vector.memset(c23[:, :, S2:S2P], 0.0)
                    for b in range(SUBB):
                        nc.vector.tensor_tensor_scan(
                            c2[:, b * S2P: b * S2P + S2], m2[:, b * S2P: b * S2P + S2],
                            zer262[:, 0:S2], 0.0, OP.add, OP.add)
                    nc.vector.tensor_tensor(c2[:], c2[:], m2[:], OP.mult)
                    cap2 = mpool.tile([128, SUBB * S2P], F16, tag="aneg", name="cap2")
                    nc.vector.tensor_scalar(cap2[:], c2[:], 4.5, 0.0, OP.is_lt, OP.add)
                    nc.vector.tensor_tensor(c2[:], c2[:], cap2[:], OP.mult)
                    idx2 = mpool.tile([128, SUBB, 2 * S2P], I16, tag="idxt", name="idx2")
                    c23b = c2[:].rearrange("p (b t) -> p b t", t=S2P)
                    nc.vector.tensor_scalar(idx2[:, :, 0: 2 * S2P: 2], c23b[:, :, :], 2.0, -2.0, OP.mult, OP.add)
                    nc.vector.tensor_scalar(idx2[:, :, 1: 2 * S2P: 2], c23b[:, :, :], 2.0, -1.0, OP.mult, OP.add)
                    th4 = th4pool.tile([128, SUBB * 8], F32, tag="th4", name="th4")
                    t4u = th4[:].bitcast(U16)
                    thu = th[:].bitcast(U16)
                    for b in range(SUBB):
                        dst = t4u[:, 2 * b * 8: 2 * b * 8 + 16]
                        src_ = thu[:, 2 * b * S2P: 2 * b * S2P + 2 * S2P]
                        nc.gpsimd.local_scatter(
                            dst, src_, idx2[:, b, 0: 2 * S2P],
                            channels=128, num_elems=16, num_idxs=2 * S2P)
                    th4v = th4[:].rearrange("p (b j) -> p b j", j=8)
                    fc_ps = psfc.tile([6, SUBB], F32, tag="fc_ps", name="fc_ps")
                    for j in range(4):
                        rhs = th4v[:, :, j:j + 1].rearrange("p b one -> p (b one)")
                        nc.tensor.matmul(fc_ps[:], fcw[:, (q * 4 + j) * 6:(q * 4 + j + 1) * 6], rhs,
                                         start=(j == 0), stop=(j == 3))
                    gc0 = s * SUBB
                    nc.vector.tensor_tensor(out_sb[:, gc0:gc0 + SUBB], out_sb[:, gc0:gc0 + SUBB], fc_ps[:], OP.add)
                    if s == 0 and h == 1:
                        dbg_t4g = outpool.tile([128, SUBB * 8], F32, name="dbg_t4g")
                        nc.vector.tensor_copy(dbg_t4g[:], th4[:])
                        nc.sync.dma_start(dbg_th4b[:], dbg_t4g[:])
                    if s == 0 and h == 0:
                        nc.sync.dma_start(dbg_mu[:], MU[:])
                        nc.sync.dma_start(dbg_tb[:], TB[:])
                        nc.sync.dma_start(dbg_cbl[:], CBl[:])
                        nc.sync.dma_start(dbg_vst[:], VST[:])
                        dbg_t4f = outpool.tile([128, SUBB * 8], F32, name="dbg_t4f")
                        nc.vector.tensor_copy(dbg_t4f[:], th4[:])
                        nc.sync.dma_start(dbg_th4[:], dbg_t4f[:])
            nc.sync.dma_start(out[:], out_sb[:])
        es.close()
    nc.compile()
    return nc


_CACHE = {}


def kernel(x, emb, w1, b1, w2, b2, fc_w, fc_b):
    from concourse.bass_utils import run_bass_kernel_spmd
    SUBB, NSUB, NCORES = 16, 4, 8
    B_C = SUBB * NSUB
    x = np.asarray(x)
    if 'nc' not in _CACHE:
        _CACHE['nc'] = build_nc(SUBB, NSUB)
    nc = _CACHE['nc']
    in_maps = []
    for c in range(NCORES):
        in_maps.append(host_prep(x[c * B_C:(c + 1) * B_C], emb, w1, b1, w2, b2, fc_w, SUBB, NSUB))
    res = run_bass_kernel_spmd(nc, in_maps, list(range(NCORES)))
    outs = [np.asarray(r["out"]).reshape(6, B_C).T for r in res.results]
    out = np.concatenate(outs, axis=0) + np.asarray(fc_b, np.float32)[None, :]
    return out.astype(np.float32)


# revision 21
# speedup vs baseline: 1.0419x; 1.0419x over previous
"""DCNN Trainium kernel v2: grouped conv1 + kmax131 + grouped conv2 + top4 + FC.

All-bf16 value pipeline after conv1 (f32 matmuls): per-row k-max threshold via
bf16 bisection (4x-mode tensor_scalar counts) + exact max8 closer in the bf16
domain; compaction via single-index bf16 local_scatter with idx = cs*msk - 1;
conv bias folded into Act-engine PSUM->SBUF activations; conv2/fc in bf16/f16;
top-4 selection in tanh domain.  Copies/memsets offloaded to Pool engine.
"""
import numpy as np
import concourse.bass as bass
import concourse.tile as tile
from concourse import bacc, mybir

F32 = mybir.dt.float32
F16 = mybir.dt.float16
BF16 = mybir.dt.bfloat16
I16 = mybir.dt.int16
I32 = mybir.dt.int32
U16 = mybir.dt.uint16
OP = mybir.AluOpType
AFT = mybir.ActivationFunctionType
AXL = mybir.AxisListType

S = 256
S1 = 262          # conv1 out length
K1 = 131          # layer-1 keep count
S1P = 264         # padded Vb row stride
H1W = 140         # compact row stride: 4 pad + 132 + 4 pad
S2 = 135          # conv2 out length
S2P = 136
ROUNDS = 7
BIG = 1024.0


def host_prep(x_core, emb, w1, b1, w2, b2, fc_w, SUBB, NSUB):
    B_C = SUBB * NSUB
    assert x_core.shape == (B_C, S)
    toks = np.ascontiguousarray(x_core.reshape(-1).astype(np.int32))
    ncol_s = SUBB * S // 128
    xi = toks.reshape(NSUB, ncol_s, 128).transpose(2, 0, 1).reshape(128, NSUB * ncol_s)

    w1 = np.asarray(w1, np.float32)   # (512, 1, 7)
    b1 = np.asarray(b1, np.float32)
    w2 = np.asarray(w2, np.float32)   # (512, 8, 5)
    b2 = np.asarray(b2, np.float32)
    fc_w = np.asarray(fc_w, np.float32)

    # conv1 lhsT with fold fused: l1[u, gl*7+k, rl*8+f] = w1[(16u+gl)*8+f, 0, k] if rl==(16(u%2)+gl)//2
    u_, gl_, k_, rl_, f_ = np.ix_(np.arange(4), np.arange(16), np.arange(7), np.arange(16), np.arange(8))
    val = w1[(16 * u_ + gl_) * 8 + f_, 0, k_]                       # (4,16,7,16,8)
    sel = (rl_ == (16 * (u_ % 2) + gl_) // 2)
    l1 = (val * sel).astype(np.float32).reshape(4, 112, 128)

    r_ = np.arange(32)
    f8 = np.arange(8)
    bf1 = (b1[(2 * r_[:, None]) * 8 + f8[None, :]] + b1[(2 * r_[:, None] + 1) * 8 + f8[None, :]]).reshape(256)
    bf1_t = np.ascontiguousarray(bf1.reshape(2, 128).T)             # [128, 2]

    q_, k5_, rrl_, ci_, r2l_, f2_ = np.ix_(np.arange(2), np.arange(5), np.arange(16), np.arange(8), np.arange(8), np.arange(16))
    val2 = w2[(16 * q_ + rrl_) * 16 + f2_, ci_, k5_]                # (2,5,16,8,8,16)
    sel2 = (r2l_ == rrl_ // 2)
    l2 = (val2 * sel2).astype(np.float32).reshape(10, 128, 128)
    r2_ = np.arange(16)
    f16_ = np.arange(16)
    bf2 = (b2[(2 * r2_[:, None]) * 16 + f16_[None, :]] + b2[(2 * r2_[:, None] + 1) * 16 + f16_[None, :]]).reshape(256)
    bf2_t = np.ascontiguousarray(bf2.reshape(2, 128).T)             # [128, 2]

    cl_ = np.arange(128)
    fcw = np.stack([fc_w[:, (128 * q + cl_) * 4 + j].T for q in range(2) for j in range(4)], 0)

    s1sq = (w1[:, 0, :] ** 2).sum(1)                                 # (512,)
    var = 0.01 * (s1sq[(2 * r_[:, None]) * 8 + f8[None, :]] + s1sq[(2 * r_[:, None] + 1) * 8 + f8[None, :]])
    sig = np.sqrt(var).astype(np.float32).reshape(2, 128)            # [m, rl*8+f]
    # sg[p, h*16+b] = 0.35 * sig[h, p]
    sg = np.repeat(0.35 * sig.T[:, :, None], SUBB, axis=2).reshape(128, 2 * SUBB)

    def as_bf16(a):
        # numpy has no bf16; emulate via f32 -> truncate-round mantissa
        x = np.asarray(a, np.float32).copy()
        u = x.view(np.uint32)
        u += 0x7FFF + ((u >> 16) & 1)
        return (u & 0xFFFF0000).view(np.float32).astype(np.float32)

    l2_flat = np.ascontiguousarray(l2.transpose(1, 0, 2).reshape(128, 1280))
    fcw_flat = np.ascontiguousarray(fcw.transpose(1, 0, 2).reshape(128, 48))

    return dict(
        x_idx=np.ascontiguousarray(xi),
        emb=np.ascontiguousarray(np.asarray(emb, np.float32)),
        l1=np.ascontiguousarray(l1.transpose(1, 0, 2).reshape(112, 512)),
        bf1=bf1_t,
        l2=l2_flat,
        bf2=bf2_t,
        fcw=fcw_flat,
        sg=np.ascontiguousarray(sg.astype(np.float32)),
        iota8=np.tile(np.arange(8, dtype=np.float32), (128, 1)),
        ident=np.eye(128, dtype=np.float32),
    )


def _win(ap, K, T):
    """[P, W] AP -> [P, K, T] overlapping window view (free steps 1, 1)."""
    l = [list(p) for p in ap.ap]
    assert len(l) == 2 and l[1][0] == 1
    return bass.AP(ap.tensor, ap.offset, [l[0], [1, K], [1, T]])


def build_nc(SUBB, NSUB):
    B_C = SUBB * NSUB
    ncol_s = SUBB * S // 128
    nc = bacc.Bacc("TRN2", target_bir_lowering=False, debug=False)
    D = {}
    D['x_idx'] = nc.dram_tensor("x_idx", [128, NSUB * ncol_s], I32, kind="ExternalInput")
    D['emb'] = nc.dram_tensor("emb", [100000, 64], F32, kind="ExternalInput")
    D['l1'] = nc.dram_tensor("l1", [112, 512], F32, kind="ExternalInput")
    D['bf1'] = nc.dram_tensor("bf1", [128, 2], F32, kind="ExternalInput")
    D['l2'] = nc.dram_tensor("l2", [128, 1280], F32, kind="ExternalInput")
    D['bf2'] = nc.dram_tensor("bf2", [128, 2], F32, kind="ExternalInput")
    D['fcw'] = nc.dram_tensor("fcw", [128, 48], F32, kind="ExternalInput")
    D['sg'] = nc.dram_tensor("sg", [128, 2 * SUBB], F32, kind="ExternalInput")
    D['iota8'] = nc.dram_tensor("iota8", [128, 8], F32, kind="ExternalInput")
    D['ident'] = nc.dram_tensor("ident", [128, 128], F32, kind="ExternalInput")
    out = nc.dram_tensor("out", [6, B_C], F32, kind="ExternalOutput")
    NB2 = 2 * SUBB
    with tile.TileContext(nc) as tc:
        from contextlib import ExitStack
        es = ExitStack()
        cpool = es.enter_context(tc.tile_pool(name="const", bufs=1))
        xtpool = es.enter_context(tc.tile_pool(name="xt", bufs=1))
        gpool = es.enter_context(tc.tile_pool(name="gather", bufs=1))
        ptr = es.enter_context(tc.tile_pool(name="ptr", bufs=2, space="PSUM"))
        icpool = es.enter_context(tc.tile_pool(name="ic", bufs=3))
        ps1pool = es.enter_context(tc.tile_pool(name="ps1", bufs=2, space="PSUM"))
        vspool = es.enter_context(tc.tile_pool(name="vs", bufs=2))
        stpool = es.enter_context(tc.tile_pool(name="st", bufs=1))
        scrpool = es.enter_context(tc.tile_pool(name="scr", bufs=4))
        mpool = es.enter_context(tc.tile_pool(name="mask", bufs=1))
        h1cpool = es.enter_context(tc.tile_pool(name="h1c", bufs=1))
        ps2pool = es.enter_context(tc.tile_pool(name="ps2", bufs=2, space="PSUM"))
        thpool = es.enter_context(tc.tile_pool(name="th", bufs=1))
        th4pool = es.enter_context(tc.tile_pool(name="th4", bufs=2))
        psfc = es.enter_context(tc.tile_pool(name="psfc", bufs=2, space="PSUM"))
        outpool = es.enter_context(tc.tile_pool(name="outp", bufs=1))
        if True:
            ident = cpool.tile([128, 128], F32)
            nc.gpsimd.dma_start(ident[:], D['ident'][:])
            iota8 = cpool.tile([128, 8], F32)
            nc.gpsimd.dma_start(iota8[:], D['iota8'][:])
            sg = cpool.tile([128, NB2], F32)
            nc.gpsimd.dma_start(sg[:], D['sg'][:])
            bf1 = cpool.tile([128, 2], F32)
            nc.gpsimd.dma_start(bf1[:], D['bf1'][:])
            bf2 = cpool.tile([128, 2], F32)
            nc.gpsimd.dma_start(bf2[:], D['bf2'][:])
            w1t = cpool.tile([112, 512], F32)
            nc.gpsimd.dma_start(w1t[:], D['l1'][:])
            w2t = cpool.tile([128, 1280], F32)
            nc.gpsimd.dma_start(w2t[:], D['l2'][:])
            fcw = cpool.tile([128, 48], F32)
            nc.gpsimd.dma_start(fcw[:], D['fcw'][:])
            xidx = cpool.tile([128, NSUB * ncol_s], I32)
            nc.gpsimd.dma_start(xidx[:], D['x_idx'][:])
            zer262 = cpool.tile([128, S1], F16)
            nc.vector.memset(zer262[:], 0.0)
            out_sb = outpool.tile([6, B_C], F32)
            nc.vector.memset(out_sb[:], 0.0)

            MU = stpool.tile([128, NB2], F32, name="MU")
            TA = stpool.tile([128, NB2], F32, name="TA")
            TB = stpool.tile([128, NB2], F32, name="TB")
            PV = stpool.tile([128, NB2], F32, name="PV")
            CC = stpool.tile([128, NB2], F32, name="CC")
            CBl = stpool.tile([128, NB2], F32, name="CBl")
            JJ = stpool.tile([128, NB2], F32, name="JJ")
            VST = stpool.tile([128, NB2], F32, name="VST")
            MX = stpool.tile([128, NB2 * 8], F32, name="MX")
            MX2 = stpool.tile([128, NB2 * 8], F32, name="MX2")
            for t_ in (MU, TA, TB, PV, CC, CBl, JJ, VST, MX, MX2):
                nc.vector.memset(t_[:], 0.0)

            for s in range(NSUB):
                # ---------------- PHASE A: gather + transpose ----------------
                xt = xtpool.tile([64, SUBB * 268], F32, tag="xt", name="xt")
                xt3 = xt[:].rearrange("p (b t) -> p b t", t=268)
                nc.vector.memset(xt3[:, :, 0:6], 0.0)
                nc.vector.memset(xt3[:, :, 262:268], 0.0)
                for i in range(ncol_s):
                    gt = gpool.tile([128, 64], F32, tag="gt", name="gt", bufs=2)
                    nc.gpsimd.indirect_dma_start(
                        out=gt[:], out_offset=None, in_=D['emb'][:],
                        in_offset=bass.IndirectOffsetOnAxis(
                            ap=xidx[:, s * ncol_s + i: s * ncol_s + i + 1], axis=0))
                    pt = ptr.tile([64, 128], F32, tag="pt", name="pt")
                    nc.tensor.transpose(out=pt[:], in_=gt[:], identity=ident[:])
                    off = (i // 2) * 268 + 6 + (i % 2) * 128
                    nc.scalar.activation(xt[:, off:off + 128], pt[:], AFT.Identity)

                # ---------------- PHASE B: conv1 (+fold +bias via Act) -------
                Vs = [vspool.tile([128, SUBB * S1P], F32, tag=f"Vs{h}", name=f"Vs{h}") for h in range(2)]
                for h in range(2):
                    vs3 = Vs[h][:].rearrange("p (b t) -> p b t", t=S1P)
                    nc.vector.memset(vs3[:, :, S1:S1P], 0.0)

                for b in range(SUBB):
                    ics = []
                    for u in range(4):
                        ic = icpool.tile([112, S1], F32, tag=f"ic{u}", name=f"ic{u}")
                        sap = _win(xt[16 * u:16 * u + 16, b * 268: b * 268 + 268], 7, S1)
                        nc.sync.dma_start(ic[:], sap)
                        ics.append(ic)
                    for m in range(2):
                        col = m * SUBB + b
                        p1 = ps1pool.tile([128, S1], F32, tag="p1", name="p1")
                        nc.tensor.matmul(p1[:], w1t[:, 2 * m * 128:(2 * m + 1) * 128], ics[2 * m][:],
                                         start=True, stop=False)
                        nc.tensor.matmul(p1[:], w1t[:, (2 * m + 1) * 128:(2 * m + 2) * 128], ics[2 * m + 1][:],
                                         start=False, stop=True)
                        nc.scalar.activation(Vs[m][:, b * S1P: b * S1P + S1], p1[:],
                                             AFT.Identity, bias=bf1[:, m:m + 1],
                                             accum_out=MU[:, col:col + 1])

                # ---------------- PHASE C: kmax-131 (both h jointly) ---------
                nc.vector.tensor_scalar(PV[:], MU[:], 1.0 / S1, 0.0, OP.mult, OP.add)
                nc.vector.tensor_tensor(TA[:], PV[:], sg[:], OP.subtract)
                nc.vector.tensor_tensor(TB[:], PV[:], sg[:], OP.add)
                for r in range(ROUNDS):
                    nc.vector.tensor_tensor(PV[:], TA[:], TB[:], OP.add)
                    nc.vector.tensor_scalar(PV[:], PV[:], 0.5, 0.0, OP.mult, OP.add)
                    for h in range(2):
                        for b in range(SUBB):
                            col = h * SUBB + b
                            scr = scrpool.tile([128, S1], F32, tag="scr32", name="scr32")
                            nc.vector.tensor_scalar(
                                scr[:], Vs[h][:, b * S1P: b * S1P + S1],
                                PV[:, col:col + 1], 0.0, OP.is_ge, OP.add,
                                accum_out=CC[:, col:col + 1])
                    cond = scrpool.tile([128, NB2], I32, tag="cond", name="cond")
                    condn = scrpool.tile([128, NB2], I32, tag="condn", name="condn")
                    nc.vector.tensor_scalar(cond[:], CC[:], float(K1) - 0.5, 0.0, OP.is_ge, OP.add)
                    nc.vector.tensor_scalar(condn[:], CC[:], float(K1) - 0.5, 0.0, OP.is_lt, OP.add)
                    nc.vector.copy_predicated(TA[:], cond[:], PV[:])
                    nc.vector.copy_predicated(TB[:], condn[:], PV[:])

                # final mask(<TB) + count + closer + compact + conv2, per h
                for h in range(2):
                    mlt = mpool.tile([128, SUBB * S1P], F16, tag="mlt", name="mlt")
                    m3 = mlt[:].rearrange("p (b t) -> p b t", t=S1P)
                    nc.vector.memset(m3[:, :, S1:S1P], 0.0)
                    for b in range(SUBB):
                        col = h * SUBB + b
                        nc.vector.tensor_scalar(
                            mlt[:, b * S1P: b * S1P + S1], Vs[h][:, b * S1P: b * S1P + S1],
                            TB[:, col:col + 1], 0.0, OP.is_lt, OP.add,
                            accum_out=CBl[:, col:col + 1])
                    c0, c1 = h * SUBB, (h + 1) * SUBB
                    nc.vector.tensor_scalar(JJ[:, c0:c1], CBl[:, c0:c1], 1.0,
                                            -(float(S1 - K1 + 1)), OP.mult, OP.add)
                    # exclude >=TB cells by pushing them to ~-BIG (in-place on Vs)
                    aneg = mpool.tile([128, SUBB * S1P], F16, tag="aneg", name="aneg")
                    nc.vector.tensor_scalar(aneg[:], mlt[:], BIG, -BIG, OP.mult, OP.add)
                    msel = mpool.tile([128, SUBB * S1P], F32, tag="mlt", name="msel")
                    nc.vector.tensor_tensor(msel[:], Vs[h][:], aneg[:], OP.add)
                    for b in range(SUBB):
                        col = h * SUBB + b
                        nc.vector.max(MX[:, col * 8: col * 8 + 8], msel[:, b * S1P: b * S1P + S1])
                    # VST = jj-th largest below TB (one-hot via iota8)
                    oh = scrpool.tile([128, SUBB, 8], F32, tag="oh", name="oh")
                    jj_b = JJ[:, c0:c1].unsqueeze(2).broadcast_to([128, SUBB, 8])
                    io_b = iota8[:].unsqueeze(1).broadcast_to([128, SUBB, 8])
                    nc.vector.tensor_tensor(oh[:], io_b, jj_b, OP.is_equal)
                    mx3 = MX[:, c0 * 8: c1 * 8].rearrange("p (c e) -> p c e", e=8)
                    nc.vector.tensor_tensor(oh[:], oh[:], mx3, OP.mult)
                    nc.vector.tensor_reduce(VST[:, c0:c1], oh[:], AXL.X, OP.add)

                    H1C = h1cpool.tile([128, SUBB * H1W], F32, tag=f"H1C{h}", name=f"H1C{h}")
                    nc.gpsimd.memset(H1C[:], 0.0)
                    msk = mpool.tile([128, SUBB * S1P], F16, tag="msk", name="msk")
                    mk3 = msk[:].rearrange("p (b t) -> p b t", t=S1P)
                    nc.vector.memset(mk3[:, :, S1:S1P], 0.0)
                    for b in range(SUBB):
                        col = h * SUBB + b
                        nc.vector.tensor_scalar(
                            msk[:, b * S1P: b * S1P + S1], Vs[h][:, b * S1P: b * S1P + S1],
                            VST[:, col:col + 1], 0.0, OP.is_ge, OP.add)
                    cs = mpool.tile([128, SUBB * S1P], F16, tag="cs", name="cs")
                    cs3 = cs[:].rearrange("p (b t) -> p b t", t=S1P)
                    nc.vector.memset(cs3[:, :, S1:S1P], 0.0)
                    for b in range(SUBB):
                        nc.vector.tensor_tensor_scan(
                            cs[:, b * S1P: b * S1P + S1], msk[:, b * S1P: b * S1P + S1],
                            zer262[:], 0.0, OP.add, OP.add)
                    nc.vector.tensor_tensor(cs[:], cs[:], msk[:], OP.mult)
                    capc = mpool.tile([128, SUBB * S1P], F16, tag="aneg", name="capc")
                    nc.vector.tensor_scalar(capc[:], cs[:], float(K1) + 0.5, 0.0, OP.is_lt, OP.add)
                    nc.vector.tensor_tensor(cs[:], cs[:], capc[:], OP.mult)
                    idxt = mpool.tile([128, SUBB, 2 * S1P], I16, tag="idxt", name="idxt")
                    cs3b = cs[:].rearrange("p (b t) -> p b t", t=S1P)
                    nc.vector.tensor_scalar(idxt[:, :, 0: 2 * S1: 2], cs3b[:, :, 0:S1], 2.0, -2.0, OP.mult, OP.add)
                    nc.vector.tensor_scalar(idxt[:, :, 1: 2 * S1: 2], cs3b[:, :, 0:S1], 2.0, -1.0, OP.mult, OP.add)
                    h1u = H1C[:].bitcast(U16)
                    vsu = Vs[h][:].bitcast(U16)
                    for b in range(SUBB):
                        dst = h1u[:, 2 * (b * H1W + 4): 2 * (b * H1W + 136)]
                        src_ = vsu[:, 2 * b * S1P: 2 * b * S1P + 2 * S1]
                        nc.gpsimd.local_scatter(
                            dst, src_, idxt[:, b, 0: 2 * S1],
                            channels=128, num_elems=264, num_idxs=2 * S1)
                    nc.scalar.activation(H1C[:], H1C[:], AFT.Tanh)

                    # ------------- PHASE D: conv2 + top4 + fc (q == h) -------
                    q = h
                    th = thpool.tile([128, SUBB * S2P], F32, tag="th", name="th")
                    th3 = th[:].rearrange("p (b t) -> p b t", t=S2P)
                    nc.vector.memset(th3[:, :, S2:S2P], 0.0)
                    m2 = mpool.tile([128, SUBB * S2P], F16, tag="msk", name="m2")
                    m23 = m2[:].rearrange("p (b t) -> p b t", t=S2P)
                    nc.vector.memset(m23[:, :, S2:S2P], 0.0)
                    for b in range(SUBB):
                        col = q * SUBB + b
                        p2 = ps2pool.tile([128, S2], F32, tag="p2", name="p2")
                        for k in range(5):
                            nc.tensor.matmul(p2[:], w2t[:, (q * 5 + k) * 128:(q * 5 + k + 1) * 128],
                                             H1C[:, b * H1W + k: b * H1W + k + S2],
                                             start=(k == 0), stop=(k == 4))
                        nc.scalar.activation(th[:, b * S2P: b * S2P + S2], p2[:],
                                             AFT.Tanh, bias=bf2[:, q:q + 1])
                        # top-4 selection on exact f32 psum (monotone under bias+tanh)
                        nc.vector.max(MX2[:, col * 8: col * 8 + 8], p2[:])
                        nc.vector.tensor_scalar(
                            m2[:, b * S2P: b * S2P + S2], p2[:],
                            MX2[:, col * 8 + 3: col * 8 + 4], 0.0, OP.is_ge, OP.add)
                    c2 = mpool.tile([128, SUBB * S2P], F16, tag="cs", name="c2")
                    c23 = c2[:].rearrange("p (b t) -> p b t", t=S2P)
                    nc.vector.memset(c23[:, :, S2:S2P], 0.0)
                    for b in range(SUBB):
                        nc.vector.tensor_tensor_scan(
                            c2[:, b * S2P: b * S2P + S2], m2[:, b * S2P: b * S2P + S2],
                            zer262[:, 0:S2], 0.0, OP.add, OP.add)
                    nc.vector.tensor_tensor(c2[:], c2[:], m2[:], OP.mult)
                    cap2 = mpool.tile([128, SUBB * S2P], F16, tag="aneg", name="cap2")
                    nc.vector.tensor_scalar(cap2[:], c2[:], 4.5, 0.0, OP.is_lt, OP.add)
                    nc.vector.tensor_tensor(c2[:], c2[:], cap2[:], OP.mult)
                    idx2 = mpool.tile([128, SUBB, 2 * S2P], I16, tag="idxt", name="idx2")
                    c23b = c2[:].rearrange("p (b t) -> p b t", t=S2P)
                    nc.vector.tensor_scalar(idx2[:, :, 0: 2 * S2P: 2], c23b[:, :, :], 2.0, -2.0, OP.mult, OP.add)
                    nc.vector.tensor_scalar(idx2[:, :, 1: 2 * S2P: 2], c23b[:, :, :], 2.0, -1.0, OP.mult, OP.add)
                    th4 = th4pool.tile([128, SUBB * 8], F32, tag="th4", name="th4")
                    t4u = th4[:].bitcast(U16)
                    thu = th[:].bitcast(U16)
                    for b in range(SUBB):
                        dst = t4u[:, 2 * b * 8: 2 * b * 8 + 16]
                        src_ = thu[:, 2 * b * S2P: 2 * b * S2P + 2 * S2P]
                        nc.gpsimd.local_scatter(
                            dst, src_, idx2[:, b, 0: 2 * S2P],
                            channels=128, num_elems=16, num_idxs=2 * S2P)
                    th4v = th4[:].rearrange("p (b j) -> p b j", j=8)
                    fc_ps = psfc.tile([6, SUBB], F32, tag="fc_ps", name="fc_ps")
                    for j in range(4):
                        rhs = th4v[:, :, j:j + 1].rearrange("p b one -> p (b one)")
                        nc.tensor.matmul(fc_ps[:], fcw[:, (q * 4 + j) * 6:(q * 4 + j + 1) * 6], rhs,
                                         start=(j == 0), stop=(j == 3))
                    gc0 = s * SUBB
                    nc.vector.tensor_tensor(out_sb[:, gc0:gc0 + SUBB], out_sb[:, gc0:gc0 + SUBB], fc_ps[:], OP.add)
            nc.sync.dma_start(out[:], out_sb[:])
        es.close()
    nc.compile()
    return nc


_CACHE = {}


def kernel(x, emb, w1, b1, w2, b2, fc_w, fc_b):
    from concourse.bass_utils import run_bass_kernel_spmd
    SUBB, NSUB, NCORES = 16, 4, 8
    B_C = SUBB * NSUB
    x = np.asarray(x)
    if 'nc' not in _CACHE:
        _CACHE['nc'] = build_nc(SUBB, NSUB)
    nc = _CACHE['nc']
    in_maps = []
    for c in range(NCORES):
        in_maps.append(host_prep(x[c * B_C:(c + 1) * B_C], emb, w1, b1, w2, b2, fc_w, SUBB, NSUB))
    res = run_bass_kernel_spmd(nc, in_maps, list(range(NCORES)))
    outs = [np.asarray(r["out"]).reshape(6, B_C).T for r in res.results]
    out = np.concatenate(outs, axis=0) + np.asarray(fc_b, np.float32)[None, :]
    return out.astype(np.float32)


# revision 23
# speedup vs baseline: 1.0729x; 1.0297x over previous
"""DCNN Trainium kernel v2: grouped conv1 + kmax131 + grouped conv2 + top4 + FC.

All-bf16 value pipeline after conv1 (f32 matmuls): per-row k-max threshold via
bf16 bisection (4x-mode tensor_scalar counts) + exact max8 closer in the bf16
domain; compaction via single-index bf16 local_scatter with idx = cs*msk - 1;
conv bias folded into Act-engine PSUM->SBUF activations; conv2/fc in bf16/f16;
top-4 selection in tanh domain.  Copies/memsets offloaded to Pool engine.
"""
import numpy as np
import concourse.bass as bass
import concourse.tile as tile
from concourse import bacc, mybir

F32 = mybir.dt.float32
F16 = mybir.dt.float16
BF16 = mybir.dt.bfloat16
I16 = mybir.dt.int16
I32 = mybir.dt.int32
U16 = mybir.dt.uint16
OP = mybir.AluOpType
AFT = mybir.ActivationFunctionType
AXL = mybir.AxisListType

S = 256
S1 = 262          # conv1 out length
K1 = 131          # layer-1 keep count
S1P = 264         # padded Vb row stride
H1W = 140         # compact row stride: 4 pad + 132 + 4 pad
S2 = 135          # conv2 out length
S2P = 136
ROUNDS = 7
BIG = 1024.0


def host_prep(x_core, emb, w1, b1, w2, b2, fc_w, SUBB, NSUB):
    B_C = SUBB * NSUB
    assert x_core.shape == (B_C, S)
    toks = np.ascontiguousarray(x_core.reshape(-1).astype(np.int32))
    ncol_s = SUBB * S // 128
    xi = toks.reshape(NSUB, ncol_s, 128).transpose(2, 0, 1).reshape(128, NSUB * ncol_s)

    w1 = np.asarray(w1, np.float32)   # (512, 1, 7)
    b1 = np.asarray(b1, np.float32)
    w2 = np.asarray(w2, np.float32)   # (512, 8, 5)
    b2 = np.asarray(b2, np.float32)
    fc_w = np.asarray(fc_w, np.float32)

    # conv1 lhsT with fold fused: l1[u, gl*7+k, rl*8+f] = w1[(16u+gl)*8+f, 0, k] if rl==(16(u%2)+gl)//2
    u_, gl_, k_, rl_, f_ = np.ix_(np.arange(4), np.arange(16), np.arange(7), np.arange(16), np.arange(8))
    val = w1[(16 * u_ + gl_) * 8 + f_, 0, k_]                       # (4,16,7,16,8)
    sel = (rl_ == (16 * (u_ % 2) + gl_) // 2)
    l1 = (val * sel).astype(np.float32).reshape(4, 112, 128)

    r_ = np.arange(32)
    f8 = np.arange(8)
    bf1 = (b1[(2 * r_[:, None]) * 8 + f8[None, :]] + b1[(2 * r_[:, None] + 1) * 8 + f8[None, :]]).reshape(256)
    bf1_t = np.ascontiguousarray(bf1.reshape(2, 128).T)             # [128, 2]

    q_, k5_, rrl_, ci_, r2l_, f2_ = np.ix_(np.arange(2), np.arange(5), np.arange(16), np.arange(8), np.arange(8), np.arange(16))
    val2 = w2[(16 * q_ + rrl_) * 16 + f2_, ci_, k5_]                # (2,5,16,8,8,16)
    sel2 = (r2l_ == rrl_ // 2)
    l2 = (val2 * sel2).astype(np.float32).reshape(10, 128, 128)
    r2_ = np.arange(16)
    f16_ = np.arange(16)
    bf2 = (b2[(2 * r2_[:, None]) * 16 + f16_[None, :]] + b2[(2 * r2_[:, None] + 1) * 16 + f16_[None, :]]).reshape(256)
    bf2_t = np.ascontiguousarray(bf2.reshape(2, 128).T)             # [128, 2]

    cl_ = np.arange(128)
    fcw = np.stack([fc_w[:, (128 * q + cl_) * 4 + j].T for q in range(2) for j in range(4)], 0)

    s1sq = (w1[:, 0, :] ** 2).sum(1)                                 # (512,)
    var = 0.01 * (s1sq[(2 * r_[:, None]) * 8 + f8[None, :]] + s1sq[(2 * r_[:, None] + 1) * 8 + f8[None, :]])
    sig = np.sqrt(var).astype(np.float32).reshape(2, 128)            # [m, rl*8+f]
    # sg[p, h*16+b] = 0.35 * sig[h, p]
    sg = np.repeat(0.35 * sig.T[:, :, None], SUBB, axis=2).reshape(128, 2 * SUBB)

    def as_bf16(a):
        # numpy has no bf16; emulate via f32 -> truncate-round mantissa
        x = np.asarray(a, np.float32).copy()
        u = x.view(np.uint32)
        u += 0x7FFF + ((u >> 16) & 1)
        return (u & 0xFFFF0000).view(np.float32).astype(np.float32)

    l2_flat = np.ascontiguousarray(l2.transpose(1, 0, 2).reshape(128, 1280))
    fcw_flat = np.ascontiguousarray(fcw.transpose(1, 0, 2).reshape(128, 48))

    return dict(
        x_idx=np.ascontiguousarray(xi),
        emb=np.ascontiguousarray(np.asarray(emb, np.float32)),
        l1=np.ascontiguousarray(l1.transpose(1, 0, 2).reshape(112, 512)),
        bf1=bf1_t,
        l2=l2_flat,
        bf2=bf2_t,
        fcw=fcw_flat,
        sg=np.ascontiguousarray(sg.astype(np.float32)),
        iota8=np.tile(np.arange(8, dtype=np.float32), (128, 1)),
        ident=np.eye(128, dtype=np.float32),
    )


def _win(ap, K, T):
    """[P, W] AP -> [P, K, T] overlapping window view (free steps 1, 1)."""
    l = [list(p) for p in ap.ap]
    assert len(l) == 2 and l[1][0] == 1
    return bass.AP(ap.tensor, ap.offset, [l[0], [1, K], [1, T]])


def build_nc(SUBB, NSUB):
    B_C = SUBB * NSUB
    ncol_s = SUBB * S // 128
    nc = bacc.Bacc("TRN2", target_bir_lowering=False, debug=False)
    D = {}
    D['x_idx'] = nc.dram_tensor("x_idx", [128, NSUB * ncol_s], I32, kind="ExternalInput")
    D['emb'] = nc.dram_tensor("emb", [100000, 64], F32, kind="ExternalInput")
    D['l1'] = nc.dram_tensor("l1", [112, 512], F32, kind="ExternalInput")
    D['bf1'] = nc.dram_tensor("bf1", [128, 2], F32, kind="ExternalInput")
    D['l2'] = nc.dram_tensor("l2", [128, 1280], F32, kind="ExternalInput")
    D['bf2'] = nc.dram_tensor("bf2", [128, 2], F32, kind="ExternalInput")
    D['fcw'] = nc.dram_tensor("fcw", [128, 48], F32, kind="ExternalInput")
    D['sg'] = nc.dram_tensor("sg", [128, 2 * SUBB], F32, kind="ExternalInput")
    D['iota8'] = nc.dram_tensor("iota8", [128, 8], F32, kind="ExternalInput")
    D['ident'] = nc.dram_tensor("ident", [128, 128], F32, kind="ExternalInput")
    out = nc.dram_tensor("out", [6, B_C], F32, kind="ExternalOutput")
    NB2 = 2 * SUBB
    with tile.TileContext(nc) as tc:
        from contextlib import ExitStack
        es = ExitStack()
        cpool = es.enter_context(tc.tile_pool(name="const", bufs=1))
        xtpool = es.enter_context(tc.tile_pool(name="xt", bufs=1))
        gpool = es.enter_context(tc.tile_pool(name="gather", bufs=1))
        ptr = es.enter_context(tc.tile_pool(name="ptr", bufs=2, space="PSUM"))
        icpool = es.enter_context(tc.tile_pool(name="ic", bufs=3))
        ps1pool = es.enter_context(tc.tile_pool(name="ps1", bufs=2, space="PSUM"))
        vspool = es.enter_context(tc.tile_pool(name="vs", bufs=2))
        stpool = es.enter_context(tc.tile_pool(name="st", bufs=1))
        scrpool = es.enter_context(tc.tile_pool(name="scr", bufs=4))
        mpool = es.enter_context(tc.tile_pool(name="mask", bufs=1))
        h1cpool = es.enter_context(tc.tile_pool(name="h1c", bufs=1))
        ps2pool = es.enter_context(tc.tile_pool(name="ps2", bufs=2, space="PSUM"))
        thpool = es.enter_context(tc.tile_pool(name="th", bufs=1))
        th4pool = es.enter_context(tc.tile_pool(name="th4", bufs=2))
        psfc = es.enter_context(tc.tile_pool(name="psfc", bufs=2, space="PSUM"))
        outpool = es.enter_context(tc.tile_pool(name="outp", bufs=1))
        if True:
            ident = cpool.tile([128, 128], F32)
            nc.gpsimd.dma_start(ident[:], D['ident'][:])
            iota8 = cpool.tile([128, 8], F32)
            nc.gpsimd.dma_start(iota8[:], D['iota8'][:])
            sg = cpool.tile([128, NB2], F32)
            nc.gpsimd.dma_start(sg[:], D['sg'][:])
            bf1 = cpool.tile([128, 2], F32)
            nc.gpsimd.dma_start(bf1[:], D['bf1'][:])
            bf2 = cpool.tile([128, 2], F32)
            nc.gpsimd.dma_start(bf2[:], D['bf2'][:])
            w1t = cpool.tile([112, 512], F32)
            nc.gpsimd.dma_start(w1t[:], D['l1'][:])
            w2t = cpool.tile([128, 1280], F32)
            nc.gpsimd.dma_start(w2t[:], D['l2'][:])
            fcw = cpool.tile([128, 48], F32)
            nc.gpsimd.dma_start(fcw[:], D['fcw'][:])
            xidx = cpool.tile([128, NSUB * ncol_s], I32)
            nc.gpsimd.dma_start(xidx[:], D['x_idx'][:])
            zer262 = cpool.tile([128, S1], F16)
            nc.vector.memset(zer262[:], 0.0)
            bm2 = cpool.tile([128, 1], F32)
            nc.vector.memset(bm2[:], -2.0)
            bm1 = cpool.tile([128, 1], F32)
            nc.vector.memset(bm1[:], -1.0)
            out_sb = outpool.tile([6, B_C], F32)
            nc.vector.memset(out_sb[:], 0.0)

            MU = stpool.tile([128, NB2], F32, name="MU")
            TA = stpool.tile([128, NB2], F32, name="TA")
            TB = stpool.tile([128, NB2], F32, name="TB")
            PV = stpool.tile([128, NB2], F32, name="PV")
            CC = stpool.tile([128, NB2], F32, name="CC")
            CBl = stpool.tile([128, NB2], F32, name="CBl")
            JJ = stpool.tile([128, NB2], F32, name="JJ")
            VST = stpool.tile([128, NB2], F32, name="VST")
            MX = stpool.tile([128, NB2 * 8], F32, name="MX")
            MX2 = stpool.tile([128, NB2 * 8], F32, name="MX2")
            for t_ in (MU, TA, TB, PV, CC, CBl, JJ, VST, MX, MX2):
                nc.vector.memset(t_[:], 0.0)

            for s in range(NSUB):
                # ---------------- PHASE A: gather + transpose ----------------
                xt = xtpool.tile([64, SUBB * 268], F32, tag="xt", name="xt")
                xt3 = xt[:].rearrange("p (b t) -> p b t", t=268)
                nc.vector.memset(xt3[:, :, 0:6], 0.0)
                nc.vector.memset(xt3[:, :, 262:268], 0.0)
                for i in range(ncol_s):
                    gt = gpool.tile([128, 64], F32, tag="gt", name="gt", bufs=2)
                    nc.gpsimd.indirect_dma_start(
                        out=gt[:], out_offset=None, in_=D['emb'][:],
                        in_offset=bass.IndirectOffsetOnAxis(
                            ap=xidx[:, s * ncol_s + i: s * ncol_s + i + 1], axis=0))
                    pt = ptr.tile([64, 128], F32, tag="pt", name="pt")
                    nc.tensor.transpose(out=pt[:], in_=gt[:], identity=ident[:])
                    off = (i // 2) * 268 + 6 + (i % 2) * 128
                    nc.scalar.activation(xt[:, off:off + 128], pt[:], AFT.Identity)

                # ---------------- PHASE B: conv1 (+fold +bias via Act) -------
                Vs = [vspool.tile([128, SUBB * S1P], F32, tag=f"Vs{h}", name=f"Vs{h}") for h in range(2)]
                for h in range(2):
                    vs3 = Vs[h][:].rearrange("p (b t) -> p b t", t=S1P)
                    nc.vector.memset(vs3[:, :, S1:S1P], 0.0)

                for b in range(SUBB):
                    ics = []
                    for u in range(4):
                        ic = icpool.tile([112, S1], F32, tag=f"ic{u}", name=f"ic{u}")
                        sap = _win(xt[16 * u:16 * u + 16, b * 268: b * 268 + 268], 7, S1)
                        nc.sync.dma_start(ic[:], sap)
                        ics.append(ic)
                    for m in range(2):
                        col = m * SUBB + b
                        p1 = ps1pool.tile([128, S1], F32, tag="p1", name="p1")
                        nc.tensor.matmul(p1[:], w1t[:, 2 * m * 128:(2 * m + 1) * 128], ics[2 * m][:],
                                         start=True, stop=False)
                        nc.tensor.matmul(p1[:], w1t[:, (2 * m + 1) * 128:(2 * m + 2) * 128], ics[2 * m + 1][:],
                                         start=False, stop=True)
                        nc.scalar.activation(Vs[m][:, b * S1P: b * S1P + S1], p1[:],
                                             AFT.Identity, bias=bf1[:, m:m + 1],
                                             accum_out=MU[:, col:col + 1])

                # ---------------- PHASE C: kmax-131 (both h jointly) ---------
                nc.vector.tensor_scalar(PV[:], MU[:], 1.0 / S1, 0.0, OP.mult, OP.add)
                nc.vector.tensor_tensor(TA[:], PV[:], sg[:], OP.subtract)
                nc.vector.tensor_tensor(TB[:], PV[:], sg[:], OP.add)
                for r in range(ROUNDS):
                    nc.vector.tensor_tensor(PV[:], TA[:], TB[:], OP.add)
                    nc.vector.tensor_scalar(PV[:], PV[:], 0.5, 0.0, OP.mult, OP.add)
                    for h in range(2):
                        for b in range(SUBB):
                            col = h * SUBB + b
                            scr = scrpool.tile([128, S1], F32, tag="scr32", name="scr32")
                            nc.vector.tensor_scalar(
                                scr[:], Vs[h][:, b * S1P: b * S1P + S1],
                                PV[:, col:col + 1], 0.0, OP.is_ge, OP.add,
                                accum_out=CC[:, col:col + 1])
                    cond = scrpool.tile([128, NB2], I32, tag="cond", name="cond")
                    condn = scrpool.tile([128, NB2], I32, tag="condn", name="condn")
                    nc.vector.tensor_scalar(cond[:], CC[:], float(K1) - 0.5, 0.0, OP.is_ge, OP.add)
                    nc.vector.tensor_scalar(condn[:], CC[:], float(K1) - 0.5, 0.0, OP.is_lt, OP.add)
                    nc.vector.copy_predicated(TA[:], cond[:], PV[:])
                    nc.vector.copy_predicated(TB[:], condn[:], PV[:])

                # final mask(<TB) + count + closer + compact + conv2, per h
                for h in range(2):
                    mlt = mpool.tile([128, SUBB * S1P], F16, tag="mlt", name="mlt")
                    m3 = mlt[:].rearrange("p (b t) -> p b t", t=S1P)
                    nc.vector.memset(m3[:, :, S1:S1P], 0.0)
                    for b in range(SUBB):
                        col = h * SUBB + b
                        nc.vector.tensor_scalar(
                            mlt[:, b * S1P: b * S1P + S1], Vs[h][:, b * S1P: b * S1P + S1],
                            TB[:, col:col + 1], 0.0, OP.is_lt, OP.add,
                            accum_out=CBl[:, col:col + 1])
                    c0, c1 = h * SUBB, (h + 1) * SUBB
                    nc.vector.tensor_scalar(JJ[:, c0:c1], CBl[:, c0:c1], 1.0,
                                            -(float(S1 - K1 + 1)), OP.mult, OP.add)
                    # exclude >=TB cells by pushing them to ~-BIG (in-place on Vs)
                    aneg = mpool.tile([128, SUBB * S1P], F16, tag="aneg", name="aneg")
                    nc.vector.tensor_scalar(aneg[:], mlt[:], BIG, -BIG, OP.mult, OP.add)
                    msel = mpool.tile([128, SUBB * S1P], F32, tag="mlt", name="msel")
                    nc.vector.tensor_tensor(msel[:], Vs[h][:], aneg[:], OP.add)
                    for b in range(SUBB):
                        col = h * SUBB + b
                        nc.vector.max(MX[:, col * 8: col * 8 + 8], msel[:, b * S1P: b * S1P + S1])
                    # VST = jj-th largest below TB (one-hot via iota8)
                    oh = scrpool.tile([128, SUBB, 8], F32, tag="oh", name="oh")
                    jj_b = JJ[:, c0:c1].unsqueeze(2).broadcast_to([128, SUBB, 8])
                    io_b = iota8[:].unsqueeze(1).broadcast_to([128, SUBB, 8])
                    nc.vector.tensor_tensor(oh[:], io_b, jj_b, OP.is_equal)
                    mx3 = MX[:, c0 * 8: c1 * 8].rearrange("p (c e) -> p c e", e=8)
                    nc.vector.tensor_tensor(oh[:], oh[:], mx3, OP.mult)
                    nc.vector.tensor_reduce(VST[:, c0:c1], oh[:], AXL.X, OP.add)

                    H1C = h1cpool.tile([128, SUBB * H1W], F32, tag=f"H1C{h}", name=f"H1C{h}")
                    nc.gpsimd.memset(H1C[:], 0.0)
                    msk = mpool.tile([128, SUBB * S1P], F16, tag="msk", name="msk")
                    mk3 = msk[:].rearrange("p (b t) -> p b t", t=S1P)
                    nc.vector.memset(mk3[:, :, S1:S1P], 0.0)
                    for b in range(SUBB):
                        col = h * SUBB + b
                        nc.vector.tensor_scalar(
                            msk[:, b * S1P: b * S1P + S1], Vs[h][:, b * S1P: b * S1P + S1],
                            VST[:, col:col + 1], 0.0, OP.is_ge, OP.add)
                    cs = mpool.tile([128, SUBB * S1P], F16, tag="cs", name="cs")
                    cs3 = cs[:].rearrange("p (b t) -> p b t", t=S1P)
                    nc.vector.memset(cs3[:, :, S1:S1P], 0.0)
                    for b in range(SUBB):
                        nc.vector.tensor_tensor_scan(
                            cs[:, b * S1P: b * S1P + S1], msk[:, b * S1P: b * S1P + S1],
                            zer262[:], 0.0, OP.add, OP.add)
                    nc.vector.tensor_tensor(cs[:], cs[:], msk[:], OP.mult)
                    capc = mpool.tile([128, SUBB * S1P], F16, tag="aneg", name="capc")
                    nc.vector.tensor_scalar(capc[:], cs[:], float(K1) + 0.5, 0.0, OP.is_lt, OP.add)
                    nc.vector.tensor_tensor(cs[:], cs[:], capc[:], OP.mult)
                    idxt = mpool.tile([128, SUBB, 2 * S1P], I16, tag="idxt", name="idxt")
                    cs3b = cs[:].rearrange("p (b t) -> p b t", t=S1P)
                    nc.scalar.activation(idxt[:, :, 0: 2 * S1: 2], cs3b[:, :, 0:S1], AFT.Identity, scale=2.0, bias=bm2[:, 0:1])
                    nc.scalar.activation(idxt[:, :, 1: 2 * S1: 2], cs3b[:, :, 0:S1], AFT.Identity, scale=2.0, bias=bm1[:, 0:1])
                    h1u = H1C[:].bitcast(U16)
                    vsu = Vs[h][:].bitcast(U16)
                    for b in range(SUBB):
                        dst = h1u[:, 2 * (b * H1W + 4): 2 * (b * H1W + 136)]
                        src_ = vsu[:, 2 * b * S1P: 2 * b * S1P + 2 * S1]
                        nc.gpsimd.local_scatter(
                            dst, src_, idxt[:, b, 0: 2 * S1],
                            channels=128, num_elems=264, num_idxs=2 * S1)
                    nc.scalar.activation(H1C[:], H1C[:], AFT.Tanh)

                    # ------------- PHASE D: conv2 + top4 + fc (q == h) -------
                    q = h
                    th = thpool.tile([128, SUBB * S2P], F32, tag="th", name="th")
                    th3 = th[:].rearrange("p (b t) -> p b t", t=S2P)
                    nc.vector.memset(th3[:, :, S2:S2P], 0.0)
                    m2 = mpool.tile([128, SUBB * S2P], F16, tag="msk", name="m2")
                    m23 = m2[:].rearrange("p (b t) -> p b t", t=S2P)
                    nc.vector.memset(m23[:, :, S2:S2P], 0.0)
                    for b in range(SUBB):
                        col = q * SUBB + b
                        p2 = ps2pool.tile([128, S2], F32, tag="p2", name="p2")
                        for k in range(5):
                            nc.tensor.matmul(p2[:], w2t[:, (q * 5 + k) * 128:(q * 5 + k + 1) * 128],
                                             H1C[:, b * H1W + k: b * H1W + k + S2],
                                             start=(k == 0), stop=(k == 4))
                        nc.scalar.activation(th[:, b * S2P: b * S2P + S2], p2[:],
                                             AFT.Tanh, bias=bf2[:, q:q + 1])
                        # top-4 selection on exact f32 psum (monotone under bias+tanh)
                        nc.vector.max(MX2[:, col * 8: col * 8 + 8], p2[:])
                        nc.vector.tensor_scalar(
                            m2[:, b * S2P: b * S2P + S2], p2[:],
                            MX2[:, col * 8 + 3: col * 8 + 4], 0.0, OP.is_ge, OP.add)
                    c2 = mpool.tile([128, SUBB * S2P], F16, tag="cs", name="c2")
                    c23 = c2[:].rearrange("p (b t) -> p b t", t=S2P)
                    nc.vector.memset(c23[:, :, S2:S2P], 0.0)
                    for b in range(SUBB):
                        nc.vector.tensor_tensor_scan(
                            c2[:, b * S2P: b * S2P + S2], m2[:, b * S2P: b * S2P + S2],
                            zer262[:, 0:S2], 0.0, OP.add, OP.add)
                    nc.vector.tensor_tensor(c2[:], c2[:], m2[:], OP.mult)
                    cap2 = mpool.tile([128, SUBB * S2P], F16, tag="aneg", name="cap2")
                    nc.vector.tensor_scalar(cap2[:], c2[:], 4.5, 0.0, OP.is_lt, OP.add)
                    nc.vector.tensor_tensor(c2[:], c2[:], cap2[:], OP.mult)
                    idx2 = mpool.tile([128, SUBB, 2 * S2P], I16, tag="idxt", name="idx2")
                    c23b = c2[:].rearrange("p (b t) -> p b t", t=S2P)
                    nc.scalar.activation(idx2[:, :, 0: 2 * S2P: 2], c23b[:, :, :], AFT.Identity, scale=2.0, bias=bm2[:, 0:1])
                    nc.scalar.activation(idx2[:, :, 1: 2 * S2P: 2], c23b[:, :, :], AFT.Identity, scale=2.0, bias=bm1[:, 0:1])
                    th4 = th4pool.tile([128, SUBB * 8], F32, tag="th4", name="th4")
                    t4u = th4[:].bitcast(U16)
                    thu = th[:].bitcast(U16)
                    for b in range(SUBB):
                        dst = t4u[:, 2 * b * 8: 2 * b * 8 + 16]
                        src_ = thu[:, 2 * b * S2P: 2 * b * S2P + 2 * S2P]
                        nc.gpsimd.local_scatter(
                            dst, src_, idx2[:, b, 0: 2 * S2P],
                            channels=128, num_elems=16, num_idxs=2 * S2P)
                    th4v = th4[:].rearrange("p (b j) -> p b j", j=8)
                    fc_ps = psfc.tile([6, SUBB], F32, tag="fc_ps", name="fc_ps")
                    for j in range(4):
                        rhs = th4v[:, :, j:j + 1].rearrange("p b one -> p (b one)")
                        nc.tensor.matmul(fc_ps[:], fcw[:, (q * 4 + j) * 6:(q * 4 + j + 1) * 6], rhs,
                                         start=(j == 0), stop=(j == 3))
                    gc0 = s * SUBB
                    nc.vector.tensor_tensor(out_sb[:, gc0:gc0 + SUBB], out_sb[:, gc0:gc0 + SUBB], fc_ps[:], OP.add)
            nc.sync.dma_start(out[:], out_sb[:])
        es.close()
    nc.compile()
    return nc


_CACHE = {}


def kernel(x, emb, w1, b1, w2, b2, fc_w, fc_b):
    from concourse.bass_utils import run_bass_kernel_spmd
    SUBB, NSUB, NCORES = 16, 4, 8
    B_C = SUBB * NSUB
    x = np.asarray(x)
    if 'nc' not in _CACHE:
        _CACHE['nc'] = build_nc(SUBB, NSUB)
    nc = _CACHE['nc']
    in_maps = []
    for c in range(NCORES):
        in_maps.append(host_prep(x[c * B_C:(c + 1) * B_C], emb, w1, b1, w2, b2, fc_w, SUBB, NSUB))
    res = run_bass_kernel_spmd(nc, in_maps, list(range(NCORES)))
    outs = [np.asarray(r["out"]).reshape(6, B_C).T for r in res.results]
    out = np.concatenate(outs, axis=0) + np.asarray(fc_b, np.float32)[None, :]
    return out.astype(np.float32)


# revision 24
# speedup vs baseline: 1.0873x; 1.0134x over previous
"""DCNN Trainium kernel v2: grouped conv1 + kmax131 + grouped conv2 + top4 + FC.

All-bf16 value pipeline after conv1 (f32 matmuls): per-row k-max threshold via
bf16 bisection (4x-mode tensor_scalar counts) + exact max8 closer in the bf16
domain; compaction via single-index bf16 local_scatter with idx = cs*msk - 1;
conv bias folded into Act-engine PSUM->SBUF activations; conv2/fc in bf16/f16;
top-4 selection in tanh domain.  Copies/memsets offloaded to Pool engine.
"""
import numpy as np
import concourse.bass as bass
import concourse.tile as tile
from concourse import bacc, mybir

F32 = mybir.dt.float32
F16 = mybir.dt.float16
BF16 = mybir.dt.bfloat16
I16 = mybir.dt.int16
I32 = mybir.dt.int32
U16 = mybir.dt.uint16
OP = mybir.AluOpType
AFT = mybir.ActivationFunctionType
AXL = mybir.AxisListType

S = 256
S1 = 262          # conv1 out length
K1 = 131          # layer-1 keep count
S1P = 264         # padded Vb row stride
H1W = 140         # compact row stride: 4 pad + 132 + 4 pad
S2 = 135          # conv2 out length
S2P = 136
ROUNDS = 6
BIG = 1024.0


def host_prep(x_core, emb, w1, b1, w2, b2, fc_w, SUBB, NSUB):
    B_C = SUBB * NSUB
    assert x_core.shape == (B_C, S)
    toks = np.ascontiguousarray(x_core.reshape(-1).astype(np.int32))
    ncol_s = SUBB * S // 128
    xi = toks.reshape(NSUB, ncol_s, 128).transpose(2, 0, 1).reshape(128, NSUB * ncol_s)

    w1 = np.asarray(w1, np.float32)   # (512, 1, 7)
    b1 = np.asarray(b1, np.float32)
    w2 = np.asarray(w2, np.float32)   # (512, 8, 5)
    b2 = np.asarray(b2, np.float32)
    fc_w = np.asarray(fc_w, np.float32)

    # conv1 lhsT with fold fused: l1[u, gl*7+k, rl*8+f] = w1[(16u+gl)*8+f, 0, k] if rl==(16(u%2)+gl)//2
    u_, gl_, k_, rl_, f_ = np.ix_(np.arange(4), np.arange(16), np.arange(7), np.arange(16), np.arange(8))
    val = w1[(16 * u_ + gl_) * 8 + f_, 0, k_]                       # (4,16,7,16,8)
    sel = (rl_ == (16 * (u_ % 2) + gl_) // 2)
    l1 = (val * sel).astype(np.float32).reshape(4, 112, 128)

    r_ = np.arange(32)
    f8 = np.arange(8)
    bf1 = (b1[(2 * r_[:, None]) * 8 + f8[None, :]] + b1[(2 * r_[:, None] + 1) * 8 + f8[None, :]]).reshape(256)
    bf1_t = np.ascontiguousarray(bf1.reshape(2, 128).T)             # [128, 2]

    q_, k5_, rrl_, ci_, r2l_, f2_ = np.ix_(np.arange(2), np.arange(5), np.arange(16), np.arange(8), np.arange(8), np.arange(16))
    val2 = w2[(16 * q_ + rrl_) * 16 + f2_, ci_, k5_]                # (2,5,16,8,8,16)
    sel2 = (r2l_ == rrl_ // 2)
    l2 = (val2 * sel2).astype(np.float32).reshape(10, 128, 128)
    r2_ = np.arange(16)
    f16_ = np.arange(16)
    bf2 = (b2[(2 * r2_[:, None]) * 16 + f16_[None, :]] + b2[(2 * r2_[:, None] + 1) * 16 + f16_[None, :]]).reshape(256)
    bf2_t = np.ascontiguousarray(bf2.reshape(2, 128).T)             # [128, 2]

    cl_ = np.arange(128)
    fcw = np.stack([fc_w[:, (128 * q + cl_) * 4 + j].T for q in range(2) for j in range(4)], 0)

    s1sq = (w1[:, 0, :] ** 2).sum(1)                                 # (512,)
    var = 0.01 * (s1sq[(2 * r_[:, None]) * 8 + f8[None, :]] + s1sq[(2 * r_[:, None] + 1) * 8 + f8[None, :]])
    sig = np.sqrt(var).astype(np.float32).reshape(2, 128)            # [m, rl*8+f]
    # sg[p, h*16+b] = 0.35 * sig[h, p]
    sg = np.repeat(0.35 * sig.T[:, :, None], SUBB, axis=2).reshape(128, 2 * SUBB)

    def as_bf16(a):
        # numpy has no bf16; emulate via f32 -> truncate-round mantissa
        x = np.asarray(a, np.float32).copy()
        u = x.view(np.uint32)
        u += 0x7FFF + ((u >> 16) & 1)
        return (u & 0xFFFF0000).view(np.float32).astype(np.float32)

    l2_flat = np.ascontiguousarray(l2.transpose(1, 0, 2).reshape(128, 1280))
    fcw_flat = np.ascontiguousarray(fcw.transpose(1, 0, 2).reshape(128, 48))

    return dict(
        x_idx=np.ascontiguousarray(xi),
        emb=np.ascontiguousarray(np.asarray(emb, np.float32)),
        l1=np.ascontiguousarray(l1.transpose(1, 0, 2).reshape(112, 512)),
        bf1=bf1_t,
        l2=l2_flat,
        bf2=bf2_t,
        fcw=fcw_flat,
        sg=np.ascontiguousarray(sg.astype(np.float32)),
        iota8=np.tile(np.arange(8, dtype=np.float32), (128, 1)),
        ident=np.eye(128, dtype=np.float32),
    )


def _win(ap, K, T):
    """[P, W] AP -> [P, K, T] overlapping window view (free steps 1, 1)."""
    l = [list(p) for p in ap.ap]
    assert len(l) == 2 and l[1][0] == 1
    return bass.AP(ap.tensor, ap.offset, [l[0], [1, K], [1, T]])


def build_nc(SUBB, NSUB):
    B_C = SUBB * NSUB
    ncol_s = SUBB * S // 128
    nc = bacc.Bacc("TRN2", target_bir_lowering=False, debug=False)
    D = {}
    D['x_idx'] = nc.dram_tensor("x_idx", [128, NSUB * ncol_s], I32, kind="ExternalInput")
    D['emb'] = nc.dram_tensor("emb", [100000, 64], F32, kind="ExternalInput")
    D['l1'] = nc.dram_tensor("l1", [112, 512], F32, kind="ExternalInput")
    D['bf1'] = nc.dram_tensor("bf1", [128, 2], F32, kind="ExternalInput")
    D['l2'] = nc.dram_tensor("l2", [128, 1280], F32, kind="ExternalInput")
    D['bf2'] = nc.dram_tensor("bf2", [128, 2], F32, kind="ExternalInput")
    D['fcw'] = nc.dram_tensor("fcw", [128, 48], F32, kind="ExternalInput")
    D['sg'] = nc.dram_tensor("sg", [128, 2 * SUBB], F32, kind="ExternalInput")
    D['iota8'] = nc.dram_tensor("iota8", [128, 8], F32, kind="ExternalInput")
    D['ident'] = nc.dram_tensor("ident", [128, 128], F32, kind="ExternalInput")
    out = nc.dram_tensor("out", [6, B_C], F32, kind="ExternalOutput")
    NB2 = 2 * SUBB
    with tile.TileContext(nc) as tc:
        from contextlib import ExitStack
        es = ExitStack()
        cpool = es.enter_context(tc.tile_pool(name="const", bufs=1))
        xtpool = es.enter_context(tc.tile_pool(name="xt", bufs=1))
        gpool = es.enter_context(tc.tile_pool(name="gather", bufs=1))
        ptr = es.enter_context(tc.tile_pool(name="ptr", bufs=2, space="PSUM"))
        icpool = es.enter_context(tc.tile_pool(name="ic", bufs=3))
        ps1pool = es.enter_context(tc.tile_pool(name="ps1", bufs=2, space="PSUM"))
        vspool = es.enter_context(tc.tile_pool(name="vs", bufs=2))
        stpool = es.enter_context(tc.tile_pool(name="st", bufs=1))
        scrpool = es.enter_context(tc.tile_pool(name="scr", bufs=4))
        mpool = es.enter_context(tc.tile_pool(name="mask", bufs=1))
        h1cpool = es.enter_context(tc.tile_pool(name="h1c", bufs=1))
        ps2pool = es.enter_context(tc.tile_pool(name="ps2", bufs=2, space="PSUM"))
        thpool = es.enter_context(tc.tile_pool(name="th", bufs=1))
        th4pool = es.enter_context(tc.tile_pool(name="th4", bufs=2))
        psfc = es.enter_context(tc.tile_pool(name="psfc", bufs=2, space="PSUM"))
        outpool = es.enter_context(tc.tile_pool(name="outp", bufs=1))
        if True:
            ident = cpool.tile([128, 128], F32)
            nc.gpsimd.dma_start(ident[:], D['ident'][:])
            iota8 = cpool.tile([128, 8], F32)
            nc.gpsimd.dma_start(iota8[:], D['iota8'][:])
            sg = cpool.tile([128, NB2], F32)
            nc.gpsimd.dma_start(sg[:], D['sg'][:])
            bf1 = cpool.tile([128, 2], F32)
            nc.gpsimd.dma_start(bf1[:], D['bf1'][:])
            bf2 = cpool.tile([128, 2], F32)
            nc.gpsimd.dma_start(bf2[:], D['bf2'][:])
            w1t = cpool.tile([112, 512], F32)
            nc.gpsimd.dma_start(w1t[:], D['l1'][:])
            w2t = cpool.tile([128, 1280], F32)
            nc.gpsimd.dma_start(w2t[:], D['l2'][:])
            fcw = cpool.tile([128, 48], F32)
            nc.gpsimd.dma_start(fcw[:], D['fcw'][:])
            xidx = cpool.tile([128, NSUB * ncol_s], I32)
            nc.gpsimd.dma_start(xidx[:], D['x_idx'][:])
            zer262 = cpool.tile([128, S1], F16)
            nc.vector.memset(zer262[:], 0.0)
            bm2 = cpool.tile([128, 1], F32)
            nc.vector.memset(bm2[:], -2.0)
            bm1 = cpool.tile([128, 1], F32)
            nc.vector.memset(bm1[:], -1.0)
            out_sb = outpool.tile([6, B_C], F32)
            nc.vector.memset(out_sb[:], 0.0)

            MU = stpool.tile([128, NB2], F32, name="MU")
            TA = stpool.tile([128, NB2], F32, name="TA")
            TB = stpool.tile([128, NB2], F32, name="TB")
            PV = stpool.tile([128, NB2], F32, name="PV")
            CC = stpool.tile([128, NB2], F32, name="CC")
            CBl = stpool.tile([128, NB2], F32, name="CBl")
            JJ = stpool.tile([128, NB2], F32, name="JJ")
            VST = stpool.tile([128, NB2], F32, name="VST")
            MX = stpool.tile([128, NB2 * 8], F32, name="MX")
            MX2 = stpool.tile([128, NB2 * 8], F32, name="MX2")
            for t_ in (MU, TA, TB, PV, CC, CBl, JJ, VST, MX, MX2):
                nc.vector.memset(t_[:], 0.0)

            for s in range(NSUB):
                # ---------------- PHASE A: gather + transpose ----------------
                xt = xtpool.tile([64, SUBB * 268], F32, tag="xt", name="xt")
                xt3 = xt[:].rearrange("p (b t) -> p b t", t=268)
                nc.vector.memset(xt3[:, :, 0:6], 0.0)
                nc.vector.memset(xt3[:, :, 262:268], 0.0)
                for i in range(ncol_s):
                    gt = gpool.tile([128, 64], F32, tag="gt", name="gt", bufs=2)
                    nc.gpsimd.indirect_dma_start(
                        out=gt[:], out_offset=None, in_=D['emb'][:],
                        in_offset=bass.IndirectOffsetOnAxis(
                            ap=xidx[:, s * ncol_s + i: s * ncol_s + i + 1], axis=0))
                    pt = ptr.tile([64, 128], F32, tag="pt", name="pt")
                    nc.tensor.transpose(out=pt[:], in_=gt[:], identity=ident[:])
                    off = (i // 2) * 268 + 6 + (i % 2) * 128
                    nc.scalar.activation(xt[:, off:off + 128], pt[:], AFT.Identity)

                # ---------------- PHASE B: conv1 (+fold +bias via Act) -------
                Vs = [vspool.tile([128, SUBB * S1P], F32, tag=f"Vs{h}", name=f"Vs{h}") for h in range(2)]
                for h in range(2):
                    vs3 = Vs[h][:].rearrange("p (b t) -> p b t", t=S1P)
                    nc.vector.memset(vs3[:, :, S1:S1P], 0.0)

                for b in range(SUBB):
                    ics = []
                    for u in range(4):
                        ic = icpool.tile([112, S1], F32, tag=f"ic{u}", name=f"ic{u}")
                        sap = _win(xt[16 * u:16 * u + 16, b * 268: b * 268 + 268], 7, S1)
                        nc.sync.dma_start(ic[:], sap)
                        ics.append(ic)
                    for m in range(2):
                        col = m * SUBB + b
                        p1 = ps1pool.tile([128, S1], F32, tag="p1", name="p1")
                        nc.tensor.matmul(p1[:], w1t[:, 2 * m * 128:(2 * m + 1) * 128], ics[2 * m][:],
                                         start=True, stop=False)
                        nc.tensor.matmul(p1[:], w1t[:, (2 * m + 1) * 128:(2 * m + 2) * 128], ics[2 * m + 1][:],
                                         start=False, stop=True)
                        nc.scalar.activation(Vs[m][:, b * S1P: b * S1P + S1], p1[:],
                                             AFT.Identity, bias=bf1[:, m:m + 1],
                                             accum_out=MU[:, col:col + 1])

                # ---------------- PHASE C: kmax-131 (both h jointly) ---------
                nc.vector.tensor_scalar(PV[:], MU[:], 1.0 / S1, 0.0, OP.mult, OP.add)
                nc.vector.tensor_tensor(TA[:], PV[:], sg[:], OP.subtract)
                nc.vector.tensor_tensor(TB[:], PV[:], sg[:], OP.add)
                for r in range(ROUNDS):
                    nc.vector.tensor_tensor(PV[:], TA[:], TB[:], OP.add)
                    nc.vector.tensor_scalar(PV[:], PV[:], 0.5, 0.0, OP.mult, OP.add)
                    for h in range(2):
                        for b in range(SUBB):
                            col = h * SUBB + b
                            scr = scrpool.tile([128, S1], F32, tag="scr32", name="scr32")
                            nc.vector.tensor_scalar(
                                scr[:], Vs[h][:, b * S1P: b * S1P + S1],
                                PV[:, col:col + 1], 0.0, OP.is_ge, OP.add,
                                accum_out=CC[:, col:col + 1])
                    cond = scrpool.tile([128, NB2], I32, tag="cond", name="cond")
                    condn = scrpool.tile([128, NB2], I32, tag="condn", name="condn")
                    nc.vector.tensor_scalar(cond[:], CC[:], float(K1) - 0.5, 0.0, OP.is_ge, OP.add)
                    nc.vector.tensor_scalar(condn[:], CC[:], float(K1) - 0.5, 0.0, OP.is_lt, OP.add)
                    nc.vector.copy_predicated(TA[:], cond[:], PV[:])
                    nc.vector.copy_predicated(TB[:], condn[:], PV[:])

                # final mask(<TB) + count + closer + compact + conv2, per h
                for h in range(2):
                    mlt = mpool.tile([128, SUBB * S1P], F16, tag="mlt", name="mlt")
                    m3 = mlt[:].rearrange("p (b t) -> p b t", t=S1P)
                    nc.vector.memset(m3[:, :, S1:S1P], 0.0)
                    for b in range(SUBB):
                        col = h * SUBB + b
                        nc.vector.tensor_scalar(
                            mlt[:, b * S1P: b * S1P + S1], Vs[h][:, b * S1P: b * S1P + S1],
                            TB[:, col:col + 1], 0.0, OP.is_lt, OP.add,
                            accum_out=CBl[:, col:col + 1])
                    c0, c1 = h * SUBB, (h + 1) * SUBB
                    nc.vector.tensor_scalar(JJ[:, c0:c1], CBl[:, c0:c1], 1.0,
                                            -(float(S1 - K1 + 1)), OP.mult, OP.add)
                    # exclude >=TB cells by pushing them to ~-BIG (in-place on Vs)
                    aneg = mpool.tile([128, SUBB * S1P], F16, tag="aneg", name="aneg")
                    nc.vector.tensor_scalar(aneg[:], mlt[:], BIG, -BIG, OP.mult, OP.add)
                    msel = mpool.tile([128, SUBB * S1P], F32, tag="mlt", name="msel")
                    nc.vector.tensor_tensor(msel[:], Vs[h][:], aneg[:], OP.add)
                    for b in range(SUBB):
                        col = h * SUBB + b
                        nc.vector.max(MX[:, col * 8: col * 8 + 8], msel[:, b * S1P: b * S1P + S1])
                    # VST = jj-th largest below TB (one-hot via iota8)
                    oh = scrpool.tile([128, SUBB, 8], F32, tag="oh", name="oh")
                    jj_b = JJ[:, c0:c1].unsqueeze(2).broadcast_to([128, SUBB, 8])
                    io_b = iota8[:].unsqueeze(1).broadcast_to([128, SUBB, 8])
                    nc.vector.tensor_tensor(oh[:], io_b, jj_b, OP.is_equal)
                    mx3 = MX[:, c0 * 8: c1 * 8].rearrange("p (c e) -> p c e", e=8)
                    nc.vector.tensor_tensor(oh[:], oh[:], mx3, OP.mult)
                    nc.vector.tensor_reduce(VST[:, c0:c1], oh[:], AXL.X, OP.add)

                    H1C = h1cpool.tile([128, SUBB * H1W], F32, tag=f"H1C{h}", name=f"H1C{h}")
                    nc.gpsimd.memset(H1C[:], 0.0)
                    msk = mpool.tile([128, SUBB * S1P], F16, tag="msk", name="msk")
                    mk3 = msk[:].rearrange("p (b t) -> p b t", t=S1P)
                    nc.vector.memset(mk3[:, :, S1:S1P], 0.0)
                    for b in range(SUBB):
                        col = h * SUBB + b
                        nc.vector.tensor_scalar(
                            msk[:, b * S1P: b * S1P + S1], Vs[h][:, b * S1P: b * S1P + S1],
                            VST[:, col:col + 1], 0.0, OP.is_ge, OP.add)
                    cs = mpool.tile([128, SUBB * S1P], F16, tag="cs", name="cs")
                    cs3 = cs[:].rearrange("p (b t) -> p b t", t=S1P)
                    nc.vector.memset(cs3[:, :, S1:S1P], 0.0)
                    for b in range(SUBB):
                        nc.vector.tensor_tensor_scan(
                            cs[:, b * S1P: b * S1P + S1], msk[:, b * S1P: b * S1P + S1],
                            zer262[:], 0.0, OP.add, OP.add)
                    nc.vector.tensor_tensor(cs[:], cs[:], msk[:], OP.mult)
                    capc = mpool.tile([128, SUBB * S1P], F16, tag="aneg", name="capc")
                    nc.vector.tensor_scalar(capc[:], cs[:], float(K1) + 0.5, 0.0, OP.is_lt, OP.add)
                    nc.vector.tensor_tensor(cs[:], cs[:], capc[:], OP.mult)
                    idxt = mpool.tile([128, SUBB, 2 * S1P], I16, tag="idxt", name="idxt")
                    cs3b = cs[:].rearrange("p (b t) -> p b t", t=S1P)
                    nc.scalar.activation(idxt[:, :, 0: 2 * S1: 2], cs3b[:, :, 0:S1], AFT.Identity, scale=2.0, bias=bm2[:, 0:1])
                    nc.scalar.activation(idxt[:, :, 1: 2 * S1: 2], cs3b[:, :, 0:S1], AFT.Identity, scale=2.0, bias=bm1[:, 0:1])
                    h1u = H1C[:].bitcast(U16)
                    vsu = Vs[h][:].bitcast(U16)
                    for b in range(SUBB):
                        dst = h1u[:, 2 * (b * H1W + 4): 2 * (b * H1W + 136)]
                        src_ = vsu[:, 2 * b * S1P: 2 * b * S1P + 2 * S1]
                        nc.gpsimd.local_scatter(
                            dst, src_, idxt[:, b, 0: 2 * S1],
                            channels=128, num_elems=264, num_idxs=2 * S1)
                    nc.scalar.activation(H1C[:], H1C[:], AFT.Tanh)

                    # ------------- PHASE D: conv2 + top4 + fc (q == h) -------
                    q = h
                    th = thpool.tile([128, SUBB * S2P], F32, tag="th", name="th")
                    th3 = th[:].rearrange("p (b t) -> p b t", t=S2P)
                    nc.vector.memset(th3[:, :, S2:S2P], 0.0)
                    m2 = mpool.tile([128, SUBB * S2P], F16, tag="msk", name="m2")
                    m23 = m2[:].rearrange("p (b t) -> p b t", t=S2P)
                    nc.vector.memset(m23[:, :, S2:S2P], 0.0)
                    for b in range(SUBB):
                        col = q * SUBB + b
                        p2 = ps2pool.tile([128, S2], F32, tag="p2", name="p2")
                        for k in range(5):
                            nc.tensor.matmul(p2[:], w2t[:, (q * 5 + k) * 128:(q * 5 + k + 1) * 128],
                                             H1C[:, b * H1W + k: b * H1W + k + S2],
                                             start=(k == 0), stop=(k == 4))
                        nc.scalar.activation(th[:, b * S2P: b * S2P + S2], p2[:],
                                             AFT.Tanh, bias=bf2[:, q:q + 1])
                        # top-4 selection on exact f32 psum (monotone under bias+tanh)
                        nc.vector.max(MX2[:, col * 8: col * 8 + 8], p2[:])
                        nc.vector.tensor_scalar(
                            m2[:, b * S2P: b * S2P + S2], p2[:],
                            MX2[:, col * 8 + 3: col * 8 + 4], 0.0, OP.is_ge, OP.add)
                    c2 = mpool.tile([128, SUBB * S2P], F16, tag="cs", name="c2")
                    c23 = c2[:].rearrange("p (b t) -> p b t", t=S2P)
                    nc.vector.memset(c23[:, :, S2:S2P], 0.0)
                    for b in range(SUBB):
                        nc.vector.tensor_tensor_scan(
                            c2[:, b * S2P: b * S2P + S2], m2[:, b * S2P: b * S2P + S2],
                            zer262[:, 0:S2], 0.0, OP.add, OP.add)
                    nc.vector.tensor_tensor(c2[:], c2[:], m2[:], OP.mult)
                    cap2 = mpool.tile([128, SUBB * S2P], F16, tag="aneg", name="cap2")
                    nc.vector.tensor_scalar(cap2[:], c2[:], 4.5, 0.0, OP.is_lt, OP.add)
                    nc.vector.tensor_tensor(c2[:], c2[:], cap2[:], OP.mult)
                    idx2 = mpool.tile([128, SUBB, 2 * S2P], I16, tag="idxt", name="idx2")
                    c23b = c2[:].rearrange("p (b t) -> p b t", t=S2P)
                    nc.scalar.activation(idx2[:, :, 0: 2 * S2P: 2], c23b[:, :, :], AFT.Identity, scale=2.0, bias=bm2[:, 0:1])
                    nc.scalar.activation(idx2[:, :, 1: 2 * S2P: 2], c23b[:, :, :], AFT.Identity, scale=2.0, bias=bm1[:, 0:1])
                    th4 = th4pool.tile([128, SUBB * 8], F32, tag="th4", name="th4")
                    t4u = th4[:].bitcast(U16)
                    thu = th[:].bitcast(U16)
                    for b in range(SUBB):
                        dst = t4u[:, 2 * b * 8: 2 * b * 8 + 16]
                        src_ = thu[:, 2 * b * S2P: 2 * b * S2P + 2 * S2P]
                        nc.gpsimd.local_scatter(
                            dst, src_, idx2[:, b, 0: 2 * S2P],
                            channels=128, num_elems=16, num_idxs=2 * S2P)
                    th4v = th4[:].rearrange("p (b j) -> p b j", j=8)
                    fc_ps = psfc.tile([6, SUBB], F32, tag="fc_ps", name="fc_ps")
                    for j in range(4):
                        rhs = th4v[:, :, j:j + 1].rearrange("p b one -> p (b one)")
                        nc.tensor.matmul(fc_ps[:], fcw[:, (q * 4 + j) * 6:(q * 4 + j + 1) * 6], rhs,
                                         start=(j == 0), stop=(j == 3))
                    gc0 = s * SUBB
                    nc.vector.tensor_tensor(out_sb[:, gc0:gc0 + SUBB], out_sb[:, gc0:gc0 + SUBB], fc_ps[:], OP.add)
            nc.sync.dma_start(out[:], out_sb[:])
        es.close()
    nc.compile()
    return nc


_CACHE = {}


def kernel(x, emb, w1, b1, w2, b2, fc_w, fc_b):
    from concourse.bass_utils import run_bass_kernel_spmd
    SUBB, NSUB, NCORES = 16, 4, 8
    B_C = SUBB * NSUB
    x = np.asarray(x)
    if 'nc' not in _CACHE:
        _CACHE['nc'] = build_nc(SUBB, NSUB)
    nc = _CACHE['nc']
    in_maps = []
    for c in range(NCORES):
        in_maps.append(host_prep(x[c * B_C:(c + 1) * B_C], emb, w1, b1, w2, b2, fc_w, SUBB, NSUB))
    res = run_bass_kernel_spmd(nc, in_maps, list(range(NCORES)))
    outs = [np.asarray(r["out"]).reshape(6, B_C).T for r in res.results]
    out = np.concatenate(outs, axis=0) + np.asarray(fc_b, np.float32)[None, :]
    return out.astype(np.float32)
